# revision 34
# baseline (speedup 1.0000x reference)
"""Trainium2 Bass kernel for a pre-LN transformer encoder block.

Problem: x[4, 2048, 512], H=8 heads, d_ff=2048, f32.
Sharding: 8 cores = (batch b, seq-half). Each core computes the block for
1024 query rows of batch b; K/V cover the full 2048-row sequence of that
batch (duplicated across the pair) so no collectives are needed. The host
permutes each core's sequence so its own 1024 queries come first.

On-core dataflow:
  LN1 stats feature-major via PE column-sum matmuls (ones lhsT) over xT and
  xT^2; stats math on a [16,128] layout; scale/bias rows broadcast from DRAM.
  xn materialized once in fp8e4 DoubleRow layout [128, 2, S]; Q/K/V/Wo/FFN1
  run as fp8 DoubleRow matmuls (2x PE throughput). Scores stay bf16
  (contraction is only 64). Softmax exp is split across the Scalar engine
  (exact Exp with fp8 output) and the Vector engine (Schraudolph bit-trick:
  fp8 bits = s*8*log2e + 56 written through an int8 bitcast), alternating
  per key-block so neither engine is the bottleneck. attn@V runs fp8
  DoubleRow with a ones-column in V so the softmax denominator falls out of
  accumulator row 64; normalization multiplies by the broadcast reciprocal
  (x64 so o fits fp8). Wo runs twice (token- and feature-major, both fp8).
  FFN1 fp8 with per-column absmax weight quant (descale folded into relu);
  FFN2 bf16 for accuracy; final residual + store.
"""

import sys
import numpy as np

sys.path.insert(0, "/opt/trn_rl_repo")

B, S, D = 4, 2048, 512
H, DK, DFF = 8, 64, 2048
SQ = S // 2
P = 128
FD = 512
EPS = 1e-6
NKT = D // P          # 4  feature tiles
NST = S // P          # 16 sequence tiles
NSQT = SQ // P        # 8  own-query tiles
NMT = DFF // P        # 16 ffn tiles
SCL = 128.0           # fp8 weight prescale
OSC = 64.0            # fp8 o prescale
A_EXP = float(128.0 / np.log(2.0))
B_EXP = 16249.6       # 127*128 - 6.9 schraudolph shift (bf16 bits)

_CACHE = {}
_TRACE = {"trace": False, "trace_cores": None}
_LAST = {"res": None}


def _np_reference(x, src_mask, Wq, bq, Wk, bk, Wv, bv, Wo, bo,
                  W1, b1, W2, b2, g1, be1, g2, be2):
    """Faithful numpy fallback (used only for off-nominal inputs)."""
    x = np.asarray(x, np.float32)

    def ln(t, g, be):
        m = t.mean(-1, keepdims=True)
        var = ((t - m) ** 2).sum(-1, keepdims=True) / (t.shape[-1] - 1)
        return g * (t - m) / (np.sqrt(var) + EPS) + be

    Bv, Sv, _ = x.shape
    xn = ln(x, g1, be1)
    q = (xn @ Wq + bq).reshape(Bv, Sv, H, DK).transpose(0, 2, 1, 3)
    k = (xn @ Wk + bk).reshape(Bv, Sv, H, DK).transpose(0, 2, 1, 3)
    v = (xn @ Wv + bv).reshape(Bv, Sv, H, DK).transpose(0, 2, 1, 3)
    s = np.einsum("bhqd,bhkd->bhqk", q, k) / np.float32(np.sqrt(DK))
    s = np.where(np.asarray(src_mask) == 0, np.float32(-1e9), s)
    s = s - s.max(-1, keepdims=True)
    p = np.exp(s)
    p = p / p.sum(-1, keepdims=True)
    o = np.einsum("bhqk,bhkd->bhqd", p, v)
    o = o.transpose(0, 2, 1, 3).reshape(Bv, Sv, D)
    x = x + o @ Wo + bo
    xn = ln(x, g2, be2)
    return (x + np.maximum(xn @ W1 + b1, 0.0) @ W2 + b2).astype(np.float32)


def _build(g1, be1, g2, be2):
    import concourse.bass as bass
    import concourse.tile as tile
    from concourse import bacc, mybir
    from concourse.masks import make_identity
    from contextlib import ExitStack

    F32 = mybir.dt.float32
    F32R = mybir.dt.float32r
    BF16 = mybir.dt.bfloat16
    F8 = mybir.dt.float8e4
    I16 = mybir.dt.int16
    AF = mybir.ActivationFunctionType
    MUL = mybir.AluOpType.mult
    ADD = mybir.AluOpType.add
    MAX = mybir.AluOpType.max
    DR = mybir.MatmulPerfMode.DoubleRow

    nc = bacc.Bacc("TRN2", target_bir_lowering=False, debug=False)

    xT = nc.dram_tensor("xT", [D, S], BF16, kind="ExternalInput").ap()
    x_tok = nc.dram_tensor("x_tok", [SQ, D], F32, kind="ExternalInput").ap()
    Wq8 = nc.dram_tensor("Wq8", [2, P, 2, D], F8, kind="ExternalInput").ap()
    Wk8 = nc.dram_tensor("Wk8", [2, P, 2, D], F8, kind="ExternalInput").ap()
    Wv8 = nc.dram_tensor("Wv8", [2, P, 2, D], F8, kind="ExternalInput").ap()
    Wo8 = nc.dram_tensor("Wo8", [2, P, 2, D], F8, kind="ExternalInput").ap()
    W18 = nc.dram_tensor("W18", [2, P, 2, DFF], F8, kind="ExternalInput").ap()
    ds1 = nc.dram_tensor("ds1", [NMT, P], F32, kind="ExternalInput").ap()
    W2b = nc.dram_tensor("W2b", [DFF, D], BF16, kind="ExternalInput").ap()

    out = nc.dram_tensor("out", [SQ, D], F32, kind="ExternalOutput").ap()
    import os
    KDBG = int(os.environ.get("KDBG", "0"))
    if KDBG:
        d_kT = nc.dram_tensor("d_kT", [P, S], BF16, kind="ExternalOutput").ap()
        d_qT = nc.dram_tensor("d_qT", [P, SQ], BF16, kind="ExternalOutput").ap()
        d_vo8 = nc.dram_tensor("d_vo8", [P, 8, 2, H * (DK + 2)], F8,
                               kind="ExternalOutput").ap()
        d_pg = nc.dram_tensor("d_pg", [P, 8, 2 * FD], F8,
                              kind="ExternalOutput").ap()
        d_oT8 = nc.dram_tensor("d_oT8", [P, 2, FD], F8,
                               kind="ExternalOutput").ap()
        d_x2 = nc.dram_tensor("d_x2", [P, D], F32, kind="ExternalOutput").ap()
        d_xn2 = nc.dram_tensor("d_xn2", [P, 2, SQ], F8,
                               kind="ExternalOutput").ap()
        d_scb = nc.dram_tensor("d_scb", [P, S], BF16,
                               kind="ExternalOutput").ap()
        d_bib = nc.dram_tensor("d_bib", [P, S], BF16,
                               kind="ExternalOutput").ap()
        d_ff = nc.dram_tensor("d_ff", [P, FD], BF16,
                              kind="ExternalOutput").ap()
        d_s2b = nc.dram_tensor("d_s2b", [P, FD], F32,
                               kind="ExternalOutput").ap()
        d_b2b = nc.dram_tensor("d_b2b", [P, FD], F32,
                               kind="ExternalOutput").ap()
        d_x2T = nc.dram_tensor("d_x2T", [P, FD], F32,
                               kind="ExternalOutput").ap()
        d_rw2 = nc.dram_tensor("d_rw2", [4, P], BF16,
                               kind="ExternalOutput").ap()
        d_xn2b = nc.dram_tensor("d_xn2b", [P, 2, SQ], F8,
                                kind="ExternalOutput").ap()

    scr_s2 = nc.dram_tensor("scr_s2", [NSQT, P], F32)
    scr_b2 = nc.dram_tensor("scr_b2", [NSQT, P], F32)
    scr_dn = nc.dram_tensor("scr_dn", [16, FD], F32R)
    scr_dn2 = nc.dram_tensor("scr_dn2", [16, FD], F32R)

    def bcast_row(src_dram, nfree):
        return bass.AP(tensor=src_dram.tensor, offset=src_dram.offset,
                       ap=[[0, P], [1, nfree]])

    with tile.TileContext(nc) as tc, ExitStack() as OU:
        res = OU.enter_context(tc.tile_pool(name="res", bufs=1))

        ident = res.tile([P, P], F32, name="ident")
        make_identity(nc, ident)
        dnT = [res.tile([65, FD], F32R, name=f"dnT{t}") for t in range(8)]
        # fp8 o in DoubleRow layout: oT8[qc][g][:, i, :] holds o-features
        # g*256+i*128+p for query chunk qc
        oT8 = [[res.tile([P, 2, FD], F8, name=f"oT8_{qc}_{g}") for g in range(2)]
               for qc in range(2)]
        xT_t = [res.tile([P, S], BF16, name=f"xT{j}") for j in range(NKT)]
        xre = [res.tile([P, D], F32, name=f"xre{sq}") for sq in range(NSQT)]

        with ExitStack() as QK:
            qkv = QK.enter_context(tc.tile_pool(name="qkv", bufs=1))
            qTz = [[qkv.tile([P, SQ], BF16, name=f"qTz{j}_{h01}")
                    for h01 in range(2)] for j in range(NKT)]
            kT = [qkv.tile([P, S], BF16, name=f"kT{j}") for j in range(NKT)]
            vo8 = qkv.tile([P, 8, 2, H * (DK + 2)], BF16, name="vo8")

            # ================= phase 1: LN1 stats + QKV =================
            with ExitStack() as P1:
                p1 = P1.enter_context(tc.tile_pool(name="p1", bufs=1))
                p1s = P1.enter_context(tc.tile_pool(name="p1s", bufs=3))
                ps_qkv = P1.enter_context(
                    tc.tile_pool(name="ps_qkv", bufs=6, space="PSUM"))

                for ch in range(2):
                    for j in range(NKT):
                        eng = nc.sync if j % 2 == 0 else nc.scalar
                        eng.dma_start(
                            xT_t[j][:, ch * SQ:(ch + 1) * SQ],
                            xT[j * P:(j + 1) * P, ch * SQ:(ch + 1) * SQ])
                Wq8_t, Wk8_t, Wv8_t = [], [], []
                for g in range(2):
                    w = p1.tile([P, 2, D], F8, name=f"Wk8_{g}")
                    nc.gpsimd.dma_start(w, Wk8[g])
                    Wk8_t.append(w)
                for g in range(2):
                    w = p1.tile([P, 2, D], F8, name=f"Wq8_{g}")
                    nc.gpsimd.dma_start(w, Wq8[g])
                    Wq8_t.append(w)
                for g in range(2):
                    w = p1.tile([P, 2, D], F8, name=f"Wv8_{g}")
                    nc.gpsimd.dma_start(w, Wv8[g])
                    Wv8_t.append(w)

                ones_bf = p1.tile([P, P], BF16, name="ones_bf")
                nc.vector.memset(ones_bf, 1.0)

                # colsum stats: sum x and sum x^2 over features, per token
                xsq = [p1.tile([P, S], BF16, name=f"xsq{j}") for j in range(NKT)]
                for j in range(NKT):
                    if j % 2 == 0:
                        nc.vector.tensor_mul(xsq[j], xT_t[j], xT_t[j])
                    else:
                        nc.scalar.square(xsq[j], xT_t[j])
                scale_b = p1.tile([P, S], BF16, name="scale_b")
                bias_b = p1.tile([P, S], BF16, name="bias_b")
                for c in range(4):
                    ps1 = ps_qkv.tile([P, FD], F32, name=f"s1_{c}", tag="qk")
                    ps2 = ps_qkv.tile([P, FD], F32, name=f"s2_{c}", tag="qk")
                    for j in range(NKT):
                        nc.tensor.matmul(ps1, ones_bf,
                                         xT_t[j][:, c * FD:(c + 1) * FD],
                                         start=(j == 0), stop=(j == NKT - 1))
                    for j in range(NKT):
                        nc.tensor.matmul(ps2, ones_bf,
                                         xsq[j][:, c * FD:(c + 1) * FD],
                                         start=(j == 0), stop=(j == NKT - 1))
                    # all 128 partitions hold identical colsum rows: do the
                    # LN math full-width, no reshape/broadcast needed
                    m_c = p1s.tile([P, FD], F32, name="m_c", tag="stm")
                    v_c = p1s.tile([P, FD], F32, name="v_c", tag="stv")
                    nc.vector.tensor_scalar_mul(m_c, ps1, 1.0 / D)
                    nc.vector.tensor_mul(v_c, m_c, ps1)
                    nc.vector.tensor_sub(v_c, ps2, v_c)
                    nc.scalar.activation(v_c, v_c, AF.Sqrt, bias=0.0,
                                         scale=1.0 / (D - 1))
                    nc.vector.tensor_scalar_add(v_c, v_c, EPS)
                    with nc.allow_low_precision(reason="ln scale recip"):
                        nc.vector.reciprocal_approx_fast(v_c, v_c)
                    scs = scale_b[:, c * FD:(c + 1) * FD]
                    nc.vector.tensor_scalar_mul(scs, v_c, float(g1))
                    nc.vector.tensor_mul(m_c, m_c, scs)
                    nc.vector.tensor_scalar(bias_b[:, c * FD:(c + 1) * FD],
                                            m_c, -1.0, float(be1),
                                            op0=MUL, op1=ADD)

                # materialize xn in fp8 DoubleRow layout
                xq8 = [p1.tile([P, 2, S], F8, name=f"xq8_{g}") for g in range(2)]
                for kt in range(NKT):
                    g, i = kt // 2, kt % 2
                    t = p1s.tile([P, S], BF16, name="xnt", tag=f"xnt{kt % 2}")
                    nc.vector.tensor_mul(t, xT_t[kt], scale_b)
                    nc.vector.tensor_add(xq8[g][:, i, :], t, bias_b)

                for j in range(NKT):
                    nc.vector.memset(qTz[j][0], 0.0)
                    nc.vector.memset(qTz[j][1], 0.0)

                # K (full seq) then Q (own half), feature-major, fp8 DoubleRow
                fix_i = 0
                qk_work = []
                for j in range(NKT):
                    for sc in range(4):
                        qk_work.append(("k", j, sc))
                    for sc in range(2):
                        qk_work.append(("q", j, sc))
                    for st in range(4 * j, 4 * j + 4):
                        qk_work.append(("v", j, st))
                for (kind, j, sc) in qk_work:
                    ps = ps_qkv.tile([P, FD], F32, name="ps_q", tag="qk")
                    if kind == "v":
                        st = sc
                        for g in range(2):
                            nc.tensor.matmul(
                                ps, xq8[g][:, :, st * P:(st + 1) * P],
                                Wv8_t[g], start=(g == 0), stop=(g == 1),
                                perf_mode=DR)
                        vv = vo8[:, st // 2, st % 2, :].rearrange(
                            "p (h c) -> p h c", c=DK + 2)
                        nc.scalar.mul(vv[:, :, 0:DK],
                                      ps.rearrange("p (h c) -> p h c", c=DK),
                                      1.0 / SCL)
                        nc.vector.memset(vv[:, :, DK:DK + 2], 1.0)
                        continue
                    Wt = Wk8_t if kind == "k" else Wq8_t
                    for g in range(2):
                        nc.tensor.matmul(
                            ps, Wt[g][:, :, j * P:(j + 1) * P],
                            xq8[g][:, :, sc * FD:(sc + 1) * FD],
                            start=(g == 0), stop=(g == 1),
                            perf_mode=DR)
                    if kind == "k":
                        dst = kT[j][:, sc * FD:(sc + 1) * FD]
                        if fix_i % 2 == 0:
                            nc.vector.tensor_scalar_mul(dst, ps, 1.0 / SCL)
                        else:
                            nc.scalar.mul(dst, ps, 1.0 / SCL)
                    else:
                        for h01 in range(2):
                            bp = 64 * h01
                            dst = qTz[j][h01][bp:bp + DK,
                                              sc * FD:(sc + 1) * FD]
                            if (fix_i + h01) % 2 == 0:
                                nc.vector.tensor_scalar_mul(
                                    dst, ps[bp:bp + DK, :], 1.0 / SCL)
                            else:
                                nc.scalar.mul(dst, ps[bp:bp + DK, :],
                                              1.0 / SCL)
                    fix_i += 1


            if KDBG:
                nc.sync.dma_start(d_kT, kT[0])
                nc.sync.dma_start(d_qT, qT[0])
                nc.sync.dma_start(d_vo8, vo8)
            # ========== phases 2+3: attention + FFN, per query-chunk ==========
            with ExitStack() as P2:
                pp = P2.enter_context(tc.tile_pool(name="pp", bufs=2))
                p2s = P2.enter_context(tc.tile_pool(name="p2s", bufs=2))
                p3 = P2.enter_context(tc.tile_pool(name="p3", bufs=1))
                p3s = P2.enter_context(tc.tile_pool(name="p3s", bufs=3))
                Wo8_t, W18_t, W2_t = [], [], []
                for g in range(2):
                    w = p3.tile([P, 2, D], F8, name=f"Wo8_{g}")
                    nc.gpsimd.dma_start(w, Wo8[g])
                    Wo8_t.append(w)
                for g in range(2):
                    w = p3.tile([P, 2, DFF], F8, name=f"W18_{g}")
                    nc.gpsimd.dma_start(w, W18[g])
                    W18_t.append(w)
                for m in range(NMT):
                    w = p3.tile([P, D], BF16, name=f"W2_{m}")
                    nc.gpsimd.dma_start(w, W2b[m * P:(m + 1) * P, :])
                    W2_t.append(w)
                ds1_sb = p3.tile([P, NMT], F32, name="ds1_sb")
                nc.sync.dma_start(ds1_sb, bass.AP(
                    tensor=ds1.tensor, offset=ds1.offset,
                    ap=[[1, P], [P, NMT]]))
                for sq in range(NSQT):
                    nc.gpsimd.dma_start(xre[sq], x_tok[sq * P:(sq + 1) * P, :])
                x2tok = [p3.tile([P, D], F32, name=f"x2t{sq}")
                         for sq in range(NSQT)]
                mv2 = p3.tile([P, 2, NSQT], F32, name="mv2")
                xn2_8 = [p3.tile([P, 2, SQ], F8, name=f"xn2_8_{g}")
                         for g in range(2)]
                oT_bf = [p3.tile([P, FD], BF16, name=f"oTb_{hp}")
                         for hp in range(4)]

                def attention(qc, ps_sc, ps_acc):
                    for hp in range(4):
                        accs = {}
                        for h01 in range(2):
                            accs[h01] = ps_acc.tile(
                                [DK + 2, FD], F32,
                                name=f"acc{h01}", tag=f"acc{h01}")
                        for g in range(8):
                            sgs = {}
                            for half in range(2):
                                kt = 2 * g + half
                                for h01 in range(2):
                                    sgc = ps_sc.tile([P, FD], F32, name="sg",
                                                     tag="sg", bufs=4)
                                    nc.tensor.matmul(
                                        sgc, kT[hp][:, kt * P:(kt + 1) * P],
                                        qTz[hp][h01][:,
                                                     qc * FD:(qc + 1) * FD])
                                    sgs[(half, h01)] = sgc
                            pgs = {}
                            for half in range(2):
                                for h01 in range(2):
                                    pgt = pp.tile([P, FD], BF16, name="pg",
                                                  tag="pg", bufs=8)
                                    if (half + h01) % 2 == 0:
                                        nc.scalar.activation(
                                            pgt, sgs[(half, h01)], AF.Exp)
                                    else:
                                        nc.vector.tensor_scalar(
                                            pgt.bitcast(I16),
                                            sgs[(half, h01)], A_EXP, B_EXP,
                                            op0=MUL, op1=ADD)
                                    pgs[(half, h01)] = pgt
                            for h01 in range(2):
                                h = 2 * hp + h01
                                for half in range(2):
                                    nc.tensor.matmul(
                                        accs[h01],
                                        vo8[:, g, half,
                                            h * (DK + 2):(h + 1) * (DK + 2)],
                                        pgs[(half, h01)],
                                        start=(g == 0 and half == 0),
                                        stop=(g == 7 and half == 1))
                        t = qc * 4 + hp
                        for h01 in range(2):
                            acc = accs[h01]
                            nc.vector.tensor_copy(
                                dnT[t][64 * h01:64 * h01 + 1, :],
                                acc[DK:DK + 1, :])
                            if h01 == 0:
                                nc.vector.tensor_copy(
                                    oT_bf[hp][0:DK, :], acc[0:DK, :])
                            else:
                                nc.scalar.copy(
                                    oT_bf[hp][DK:P, :], acc[0:DK, :])
                            nc.sync.dma_start(
                                scr_dn.ap()[2 * t + h01:2 * t + h01 + 1, :],
                                dnT[t][64 * h01:64 * h01 + 1, :])

                def normalize(qc):
                    rcp = p2s.tile([64, 64], F32R, name="rcp", tag="rcp")
                    nc.sync.dma_start(rcp, bass.AP(
                        tensor=scr_dn.ap().tensor, offset=qc * 8 * FD,
                        ap=[[64, 64], [1, 64]]))
                    with nc.allow_low_precision(reason="denom recip"):
                        nc.vector.reciprocal(rcp, rcp)
                    nc.vector.tensor_scalar_mul(rcp, rcp, OSC)
                    nc.sync.dma_start(bass.AP(
                        tensor=scr_dn2.ap().tensor, offset=qc * 8 * FD,
                        ap=[[64, 64], [1, 64]]), rcp)
                    for hp in range(4):
                        t = qc * 4 + hp
                        rb_sb = p2s.tile([P, FD], F32R, name="rbs", tag="rbs")
                        for par in range(2):
                            row = scr_dn2.ap()[2 * t + par:2 * t + par + 1, :]
                            nc.sync.dma_start(
                                rb_sb[64 * par:64 * par + 64, :],
                                bass.AP(tensor=row.tensor, offset=row.offset,
                                        ap=[[0, 64]] + row.ap[1:]))
                        g, i = hp // 2, hp % 2
                        nc.vector.tensor_mul(oT8[qc][g][:, i, :],
                                             oT_bf[hp], rb_sb)
                    if KDBG and qc == 0:
                        nc.sync.dma_start(d_oT8, oT8[0][0])

                def wo_stats(qc, psp):
                    # Wo token-major + residual + LN2 stats
                    for sl in range(4):
                        sq = qc * 4 + sl
                        ps = psp.tile([P, D], F32, name="ps_wo", tag="ffn")
                        for g in range(2):
                            nc.tensor.matmul(
                                ps, oT8[qc][g][:, :, sl * P:(sl + 1) * P],
                                Wo8_t[g], start=(g == 0), stop=(g == 1),
                                perf_mode=DR)
                        nc.vector.scalar_tensor_tensor(
                            x2tok[sq], ps, 1.0 / (SCL * OSC), xre[sq],
                            op0=MUL, op1=ADD)
                        st6b = p3s.tile([P, 6], F32, name="st6b", tag="st6b")
                        nc.vector.bn_stats(st6b, x2tok[sq])
                        nc.vector.bn_aggr(mv2[:, :, sq:sq + 1], st6b)
                        if KDBG and sq == 0:
                            nc.sync.dma_start(d_x2, x2tok[0])

                def ln2_ffn(qc, psp):
                    # LN2 scale/bias rows for this half
                    sc2 = p3s.tile([P, 4], F32, name="sc2", tag="ln2")
                    bi2 = p3s.tile([P, 4], F32, name="bi2", tag="ln2")
                    std2 = p3s.tile([P, 4], F32, name="std2", tag="ln2")
                    nc.scalar.activation(std2, mv2[:, 1, 4 * qc:4 * qc + 4],
                                         AF.Sqrt, bias=0.0,
                                         scale=float(D) / (D - 1))
                    nc.vector.tensor_scalar_add(std2, std2, EPS)
                    nc.vector.reciprocal(std2, std2)
                    nc.vector.tensor_scalar_mul(sc2, std2, float(g2))
                    nc.vector.tensor_mul(std2, mv2[:, 0, 4 * qc:4 * qc + 4], sc2)
                    nc.vector.tensor_scalar(bi2, std2, -1.0, float(be2),
                                            op0=MUL, op1=ADD)
                    s2b = p2s.tile([P, FD], F32, name="s2b", tag="s2b")
                    b2b = p2s.tile([P, FD], F32, name="b2b", tag="s2b")
                    for src, scr, dst in ((sc2, scr_s2, s2b),
                                          (bi2, scr_b2, b2b)):
                        tp2 = psp.tile([4, P], F32, name="tp2", tag="ffn")
                        nc.tensor.transpose(tp2, src, ident)
                        rw2 = p3s.tile([4, P], F32, name="rw2", tag="rw2")
                        nc.vector.tensor_copy(rw2, tp2)
                        nc.sync.dma_start(scr.ap()[4 * qc:4 * qc + 4, :], rw2)
                        nc.sync.dma_start(dst, bass.AP(
                            tensor=scr.ap().tensor,
                            offset=scr.ap().offset + qc * 4 * P,
                            ap=[[0, P], [1, FD]]))
                    if KDBG and qc == 0:
                        nc.sync.dma_start(d_s2b, s2b)
                        nc.sync.dma_start(d_b2b, b2b)
                    # Wo feature-major + residual in x^T layout + LN2 apply
                    for j in range(NKT):
                        ps = psp.tile([P, FD], F32, name="ps_woT", tag="ffn")
                        for g in range(2):
                            nc.tensor.matmul(ps, Wo8_t[g][:, :, j * P:(j + 1) * P],
                                             oT8[qc][g], start=(g == 0),
                                             stop=(g == 1), perf_mode=DR)
                        x2T = p3s.tile([P, FD], F32, name="x2T", tag="x2T",
                                       bufs=2)
                        nc.vector.scalar_tensor_tensor(
                            x2T, ps, 1.0 / (SCL * OSC),
                            xT_t[j][:, qc * FD:(qc + 1) * FD],
                            op0=MUL, op1=ADD)
                        if KDBG and qc == 0 and j == 0:
                            nc.sync.dma_start(d_x2T, x2T)
                        t2 = p3s.tile([P, FD], F32, name="t2", tag="t2",
                                      bufs=2)
                        eng = nc.gpsimd if (qc == 0 and j % 2 == 1) \
                            else nc.vector
                        eng.tensor_mul(t2, x2T, s2b)
                        g2i, i2 = j // 2, j % 2
                        eng.tensor_add(
                            xn2_8[g2i][:, i2, qc * FD:(qc + 1) * FD], t2, b2b)
                    if KDBG and qc == 1:
                        nc.sync.dma_start(d_xn2, xn2_8[0])
                    if KDBG and qc == 0:
                        nc.sync.dma_start(d_xn2b, xn2_8[0])
                    # FFN1 fp8 DoubleRow with per-column descale + relu
                    ffb = []
                    for mt in range(NMT):
                        ps = psp.tile([P, FD], F32, name="ps_f1", tag="ffn")
                        for g in range(2):
                            nc.tensor.matmul(
                                ps, W18_t[g][:, :, mt * P:(mt + 1) * P],
                                xn2_8[g][:, :, qc * FD:(qc + 1) * FD],
                                start=(g == 0), stop=(g == 1), perf_mode=DR)
                        ff = p3s.tile([P, FD], BF16, name=f"ff{mt}",
                                      tag=f"ff{mt}", bufs=1)
                        nc.scalar.activation(
                            ff, ps, AF.Relu, bias=0.0,
                            scale=ds1_sb[:, mt:mt + 1])
                        ffb.append(ff)
                        if KDBG and qc == 0 and mt == 0:
                            nc.sync.dma_start(d_ff, ff)
                    # FFN2 bf16
                    for sl in range(4):
                        sq = qc * 4 + sl
                        ps = psp.tile([P, D], F32, name="ps_f2", tag="ffn")
                        for mt in range(NMT):
                            nc.tensor.matmul(ps, ffb[mt][:, sl * P:(sl + 1) * P],
                                             W2_t[mt], start=(mt == 0),
                                             stop=(mt == NMT - 1))
                        ot = p3s.tile([P, D], F32, name="ot", tag="ot",
                                      bufs=2)
                        nc.vector.tensor_add(ot, ps, x2tok[sq])
                        nc.sync.dma_start(out[sq * P:(sq + 1) * P, :], ot)

                with ExitStack() as PA:
                    ps_sc = PA.enter_context(
                        tc.tile_pool(name="ps_sc", bufs=2, space="PSUM"))
                    ps_acc = PA.enter_context(
                        tc.tile_pool(name="ps_acc", bufs=1, space="PSUM"))
                    ps_big = PA.enter_context(
                        tc.tile_pool(name="ps_big", bufs=2, space="PSUM"))
                    attention(0, ps_sc, ps_acc)
                    normalize(0)
                    wo_stats(0, ps_big)
                    attention(1, ps_sc, ps_acc)
                    ln2_ffn(0, ps_big)
                    normalize(1)
                    wo_stats(1, ps_big)
                    ln2_ffn(1, ps_big)

    nc.compile()
    return nc


def _fast_path_ok(inputs):
    if not np.all(np.asarray(inputs["src_mask"]) != 0):
        return False
    for b in ("bq", "bk", "bv", "bo", "b1", "b2"):
        if np.any(np.asarray(inputs[b]) != 0):
            return False
    return True


def _pack_dr(W, scale):
    """Pack [K, M] weight into fp8 DoubleRow layout [K//256, 128, 2, M]."""
    import ml_dtypes
    K, M = W.shape
    Wp = (np.asarray(W, np.float32) * scale).reshape(K // 256, 2, P, M)
    Wp = Wp.transpose(0, 2, 1, 3)
    return np.ascontiguousarray(Wp.astype(ml_dtypes.float8_e4m3))


def kernel(**inputs):
    x = np.ascontiguousarray(np.asarray(inputs["x"], np.float32))
    g1 = float(np.asarray(inputs["g1"]))
    be1 = float(np.asarray(inputs["be1"]))
    g2 = float(np.asarray(inputs["g2"]))
    be2 = float(np.asarray(inputs["be2"]))

    if not _fast_path_ok(inputs):
        return _np_reference(**{k: np.asarray(v) for k, v in inputs.items()})

    from concourse.bass_utils import run_bass_kernel_spmd

    key = (g1, be1, g2, be2)
    if key not in _CACHE:
        _CACHE[key] = _build(*key)
    nc = _CACHE[key]

    import ml_dtypes
    BF = ml_dtypes.bfloat16
    scale = np.float32(1.0 / np.sqrt(DK))
    Wq8 = _pack_dr(inputs["Wq"], SCL * scale)
    Wk8 = _pack_dr(inputs["Wk"], SCL)
    Wv8 = _pack_dr(inputs["Wv"], SCL)
    Wo8 = _pack_dr(inputs["Wo"], SCL)
    W1 = np.asarray(inputs["W1"], np.float32)
    scl1 = 240.0 / np.abs(W1).max(0)
    W18 = _pack_dr(W1 * scl1, 1.0)
    ds1 = np.ascontiguousarray((1.0 / scl1).reshape(NMT, P).astype(np.float32))
    W2b = np.ascontiguousarray(np.asarray(inputs["W2"], np.float32).astype(BF))

    in_maps = []
    for c in range(8):
        b, hh = c // 2, c % 2
        if hh == 0:
            xp = x[b]
        else:
            xp = np.concatenate([x[b, SQ:], x[b, :SQ]], axis=0)
        xp = np.ascontiguousarray(xp)
        in_maps.append(dict(
            xT=np.ascontiguousarray(xp.T.astype(BF)),
            x_tok=np.ascontiguousarray(xp[:SQ]),
            Wq8=Wq8, Wk8=Wk8, Wv8=Wv8, Wo8=Wo8, W18=W18, ds1=ds1, W2b=W2b))

    res = run_bass_kernel_spmd(nc, in_maps, core_ids=list(range(8)),
                               trace=_TRACE["trace"],
                               trace_cores=_TRACE["trace_cores"])
    _LAST["res"] = res

    full = np.empty((B, S, D), np.float32)
    for c in range(8):
        b, hh = c // 2, c % 2
        full[b, hh * SQ:(hh + 1) * SQ] = res.results[c]["out"]
    return full


# revision 35
# speedup vs baseline: 1.1815x; 1.1815x over previous
"""Trainium2 Bass kernel for a pre-LN transformer encoder block.

Problem: x[4, 2048, 512], H=8 heads, d_ff=2048, f32.
Sharding: 8 cores = (batch b, seq-half). Each core computes the block for
1024 query rows of batch b; K/V cover the full 2048-row sequence of that
batch (duplicated across the pair) so no collectives are needed. The host
permutes each core's sequence so its own 1024 queries come first.

On-core dataflow:
  LN1 stats feature-major via PE column-sum matmuls (ones lhsT) over xT and
  xT^2; stats math on a [16,128] layout; scale/bias rows broadcast from DRAM.
  xn materialized once in fp8e4 DoubleRow layout [128, 2, S]; Q/K/V/Wo/FFN1
  run as fp8 DoubleRow matmuls (2x PE throughput). Scores stay bf16
  (contraction is only 64). Softmax exp is split across the Scalar engine
  (exact Exp with fp8 output) and the Vector engine (Schraudolph bit-trick:
  fp8 bits = s*8*log2e + 56 written through an int8 bitcast), alternating
  per key-block so neither engine is the bottleneck. attn@V runs fp8
  DoubleRow with a ones-column in V so the softmax denominator falls out of
  accumulator row 64; normalization multiplies by the broadcast reciprocal
  (x64 so o fits fp8). Wo runs twice (token- and feature-major, both fp8).
  FFN1 fp8 with per-column absmax weight quant (descale folded into relu);
  FFN2 bf16 for accuracy; final residual + store.
"""

import sys
import numpy as np

sys.path.insert(0, "/opt/trn_rl_repo")

B, S, D = 4, 2048, 512
H, DK, DFF = 8, 64, 2048
SQ = S // 2
P = 128
FD = 512
EPS = 1e-6
NKT = D // P          # 4  feature tiles
NST = S // P          # 16 sequence tiles
NSQT = SQ // P        # 8  own-query tiles
NMT = DFF // P        # 16 ffn tiles
SCL = 128.0           # fp8 weight prescale
OSC = 64.0            # fp8 o prescale
A_EXP = float(128.0 / np.log(2.0))
B_EXP = 16249.6       # 127*128 - 6.9 schraudolph shift (bf16 bits)

_CACHE = {}
_TRACE = {"trace": False, "trace_cores": None}
_LAST = {"res": None}


def _np_reference(x, src_mask, Wq, bq, Wk, bk, Wv, bv, Wo, bo,
                  W1, b1, W2, b2, g1, be1, g2, be2):
    """Faithful numpy fallback (used only for off-nominal inputs)."""
    x = np.asarray(x, np.float32)

    def ln(t, g, be):
        m = t.mean(-1, keepdims=True)
        var = ((t - m) ** 2).sum(-1, keepdims=True) / (t.shape[-1] - 1)
        return g * (t - m) / (np.sqrt(var) + EPS) + be

    Bv, Sv, _ = x.shape
    xn = ln(x, g1, be1)
    q = (xn @ Wq + bq).reshape(Bv, Sv, H, DK).transpose(0, 2, 1, 3)
    k = (xn @ Wk + bk).reshape(Bv, Sv, H, DK).transpose(0, 2, 1, 3)
    v = (xn @ Wv + bv).reshape(Bv, Sv, H, DK).transpose(0, 2, 1, 3)
    s = np.einsum("bhqd,bhkd->bhqk", q, k) / np.float32(np.sqrt(DK))
    s = np.where(np.asarray(src_mask) == 0, np.float32(-1e9), s)
    s = s - s.max(-1, keepdims=True)
    p = np.exp(s)
    p = p / p.sum(-1, keepdims=True)
    o = np.einsum("bhqk,bhkd->bhqd", p, v)
    o = o.transpose(0, 2, 1, 3).reshape(Bv, Sv, D)
    x = x + o @ Wo + bo
    xn = ln(x, g2, be2)
    return (x + np.maximum(xn @ W1 + b1, 0.0) @ W2 + b2).astype(np.float32)


def _build(g1, be1, g2, be2):
    import concourse.bass as bass
    import concourse.tile as tile
    from concourse import bacc, mybir
    from concourse.masks import make_identity
    from contextlib import ExitStack

    F32 = mybir.dt.float32
    F32R = mybir.dt.float32r
    BF16 = mybir.dt.bfloat16
    F8 = mybir.dt.float8e4
    I16 = mybir.dt.int16
    AF = mybir.ActivationFunctionType
    MUL = mybir.AluOpType.mult
    ADD = mybir.AluOpType.add
    MAX = mybir.AluOpType.max
    DR = mybir.MatmulPerfMode.DoubleRow

    nc = bacc.Bacc("TRN2", target_bir_lowering=False, debug=False)

    xT = nc.dram_tensor("xT", [D, S], BF16, kind="ExternalInput").ap()
    x_tok = nc.dram_tensor("x_tok", [SQ, D], F32, kind="ExternalInput").ap()
    Wq8 = nc.dram_tensor("Wq8", [2, P, 2, D], F8, kind="ExternalInput").ap()
    Wk8 = nc.dram_tensor("Wk8", [2, P, 2, D], F8, kind="ExternalInput").ap()
    Wv8 = nc.dram_tensor("Wv8", [2, P, 2, D], F8, kind="ExternalInput").ap()
    Wo8 = nc.dram_tensor("Wo8", [2, P, 2, D], F8, kind="ExternalInput").ap()
    W18 = nc.dram_tensor("W18", [2, P, 2, DFF], F8, kind="ExternalInput").ap()
    ds1 = nc.dram_tensor("ds1", [NMT, P], F32, kind="ExternalInput").ap()
    W2b = nc.dram_tensor("W2b", [DFF, D], BF16, kind="ExternalInput").ap()

    out = nc.dram_tensor("out", [SQ, D], F32, kind="ExternalOutput").ap()
    scr_st = nc.dram_tensor("scr_st", [2, S], F32)
    scr_r1 = nc.dram_tensor("scr_r1", [2, S], BF16)
    import os
    KDBG = int(os.environ.get("KDBG", "0"))
    if KDBG:
        d_kT = nc.dram_tensor("d_kT", [P, S], BF16, kind="ExternalOutput").ap()
        d_qT = nc.dram_tensor("d_qT", [P, SQ], BF16, kind="ExternalOutput").ap()
        d_vo8 = nc.dram_tensor("d_vo8", [P, 8, 2, H * (DK + 2)], F8,
                               kind="ExternalOutput").ap()
        d_pg = nc.dram_tensor("d_pg", [P, 8, 2 * FD], F8,
                              kind="ExternalOutput").ap()
        d_oT8 = nc.dram_tensor("d_oT8", [P, 2, FD], F8,
                               kind="ExternalOutput").ap()
        d_x2 = nc.dram_tensor("d_x2", [P, D], F32, kind="ExternalOutput").ap()
        d_xn2 = nc.dram_tensor("d_xn2", [P, 2, SQ], F8,
                               kind="ExternalOutput").ap()
        d_scb = nc.dram_tensor("d_scb", [P, S], BF16,
                               kind="ExternalOutput").ap()
        d_bib = nc.dram_tensor("d_bib", [P, S], BF16,
                               kind="ExternalOutput").ap()
        d_ff = nc.dram_tensor("d_ff", [P, FD], BF16,
                              kind="ExternalOutput").ap()
        d_s2b = nc.dram_tensor("d_s2b", [P, FD], F32,
                               kind="ExternalOutput").ap()
        d_b2b = nc.dram_tensor("d_b2b", [P, FD], F32,
                               kind="ExternalOutput").ap()
        d_x2T = nc.dram_tensor("d_x2T", [P, FD], F32,
                               kind="ExternalOutput").ap()
        d_rw2 = nc.dram_tensor("d_rw2", [4, P], BF16,
                               kind="ExternalOutput").ap()
        d_xn2b = nc.dram_tensor("d_xn2b", [P, 2, SQ], F8,
                                kind="ExternalOutput").ap()

    scr_s2 = nc.dram_tensor("scr_s2", [NSQT, P], F32)
    scr_b2 = nc.dram_tensor("scr_b2", [NSQT, P], F32)
    scr_dn = nc.dram_tensor("scr_dn", [16, FD], F32R)
    scr_dn2 = nc.dram_tensor("scr_dn2", [16, FD], F32R)

    def bcast_row(src_dram, nfree):
        return bass.AP(tensor=src_dram.tensor, offset=src_dram.offset,
                       ap=[[0, P], [1, nfree]])

    with tile.TileContext(nc) as tc, ExitStack() as OU:
        res = OU.enter_context(tc.tile_pool(name="res", bufs=1))

        ident = res.tile([P, P], F32, name="ident")
        make_identity(nc, ident)
        dnT = [res.tile([65, FD], F32R, name=f"dnT{t}") for t in range(8)]
        # fp8 o in DoubleRow layout: oT8[qc][g][:, i, :] holds o-features
        # g*256+i*128+p for query chunk qc
        oT8 = [[res.tile([P, 2, FD], F8, name=f"oT8_{qc}_{g}") for g in range(2)]
               for qc in range(2)]
        xT_t = [res.tile([P, S], BF16, name=f"xT{j}") for j in range(NKT)]
        xre = [res.tile([P, D], F32, name=f"xre{sq}") for sq in range(NSQT)]

        with ExitStack() as QK:
            qkv = QK.enter_context(tc.tile_pool(name="qkv", bufs=1))
            qTz = [[qkv.tile([P, SQ], BF16, name=f"qTz{j}_{h01}")
                    for h01 in range(2)] for j in range(NKT)]
            kT = [qkv.tile([P, S], BF16, name=f"kT{j}") for j in range(NKT)]
            vo8 = qkv.tile([P, 8, 2, H * (DK + 2)], BF16, name="vo8")

            # ================= phase 1: LN1 stats + QKV =================
            with ExitStack() as P1:
                p1 = P1.enter_context(tc.tile_pool(name="p1", bufs=1))
                p1s = P1.enter_context(tc.tile_pool(name="p1s", bufs=3))
                ps_qkv = P1.enter_context(
                    tc.tile_pool(name="ps_qkv", bufs=6, space="PSUM"))

                for ch in range(2):
                    for j in range(NKT):
                        eng = nc.sync if j % 2 == 0 else nc.scalar
                        eng.dma_start(
                            xT_t[j][:, ch * SQ:(ch + 1) * SQ],
                            xT[j * P:(j + 1) * P, ch * SQ:(ch + 1) * SQ])
                Wq8_t, Wk8_t, Wv8_t = [], [], []
                for g in range(2):
                    w = p1.tile([P, 2, D], F8, name=f"Wk8_{g}")
                    nc.gpsimd.dma_start(w, Wk8[g])
                    Wk8_t.append(w)
                for g in range(2):
                    w = p1.tile([P, 2, D], F8, name=f"Wq8_{g}")
                    nc.gpsimd.dma_start(w, Wq8[g])
                    Wq8_t.append(w)
                for g in range(2):
                    w = p1.tile([P, 2, D], F8, name=f"Wv8_{g}")
                    nc.gpsimd.dma_start(w, Wv8[g])
                    Wv8_t.append(w)

                ones_bf = p1.tile([P, P], BF16, name="ones_bf")
                nc.vector.memset(ones_bf, 1.0)

                # colsum stats: sum x and sum x^2 over features, per token
                xsq = [p1.tile([P, S], BF16, name=f"xsq{j}") for j in range(NKT)]
                for j in range(NKT):
                    if j % 2 == 0:
                        nc.vector.tensor_mul(xsq[j], xT_t[j], xT_t[j])
                    else:
                        nc.scalar.square(xsq[j], xT_t[j])
                st_sa = p1.tile([1, S], F32, name="st_sa")
                st_sq = p1.tile([1, S], F32, name="st_sq")
                for c in range(4):
                    ps1 = ps_qkv.tile([P, FD], F32, name=f"s1_{c}", tag="qk")
                    ps2 = ps_qkv.tile([P, FD], F32, name=f"s2_{c}", tag="qk")
                    for j in range(NKT):
                        nc.tensor.matmul(ps1, ones_bf,
                                         xT_t[j][:, c * FD:(c + 1) * FD],
                                         start=(j == 0), stop=(j == NKT - 1))
                    for j in range(NKT):
                        nc.tensor.matmul(ps2, ones_bf,
                                         xsq[j][:, c * FD:(c + 1) * FD],
                                         start=(j == 0), stop=(j == NKT - 1))
                    nc.vector.tensor_copy(st_sa[0:1, c * FD:(c + 1) * FD],
                                          ps1[0:1, :])
                    nc.scalar.copy(st_sq[0:1, c * FD:(c + 1) * FD],
                                   ps2[0:1, :])
                nc.sync.dma_start(scr_st.ap()[0:1, :], st_sa)
                nc.sync.dma_start(scr_st.ap()[1:2, :], st_sq)

                def row16(scr, row):
                    return bass.AP(tensor=scr.ap().tensor,
                                   offset=scr.ap().offset + row * S,
                                   ap=[[P, 16], [1, P]])

                s16a = p1.tile([16, P], F32, name="s16a")
                s16b = p1.tile([16, P], F32, name="s16b")
                nc.sync.dma_start(s16a, row16(scr_st, 0))
                nc.sync.dma_start(s16b, row16(scr_st, 1))
                m16 = p1.tile([16, P], F32, name="m16")
                t16 = p1.tile([16, P], F32, name="t16")
                sd16 = p1.tile([16, P], F32, name="sd16")
                sc16 = p1.tile([16, P], BF16, name="sc16")
                bi16 = p1.tile([16, P], BF16, name="bi16")
                nc.vector.tensor_scalar_mul(m16, s16a, 1.0 / D)
                nc.vector.tensor_mul(t16, m16, s16a)
                nc.vector.tensor_sub(t16, s16b, t16)
                nc.scalar.activation(sd16, t16, AF.Sqrt, bias=0.0,
                                     scale=1.0 / (D - 1))
                nc.vector.tensor_scalar_add(sd16, sd16, EPS)
                nc.vector.reciprocal(sd16, sd16)
                nc.vector.tensor_scalar_mul(sc16, sd16, float(g1))
                nc.vector.tensor_mul(t16, m16, sc16)
                nc.vector.tensor_scalar(bi16, t16, -1.0, float(be1),
                                        op0=MUL, op1=ADD)

                def row16w(scr, row):
                    return bass.AP(tensor=scr.ap().tensor,
                                   offset=scr.ap().offset + row * S,
                                   ap=[[P, 16], [1, P]])

                nc.sync.dma_start(row16w(scr_r1, 0), sc16)
                nc.sync.dma_start(row16w(scr_r1, 1), bi16)
                scale_b = p1.tile([P, S], BF16, name="scale_b")
                bias_b = p1.tile([P, S], BF16, name="bias_b")
                nc.sync.dma_start(scale_b, bcast_row(scr_r1.ap()[0:1, :], S))
                nc.sync.dma_start(bias_b, bcast_row(scr_r1.ap()[1:2, :], S))

                # materialize xn in fp8 DoubleRow layout
                xq8 = [p1.tile([P, 2, S], F8, name=f"xq8_{g}") for g in range(2)]
                for kt in range(NKT):
                    g, i = kt // 2, kt % 2
                    t = p1s.tile([P, S], BF16, name="xnt", tag=f"xnt{kt % 2}")
                    nc.vector.tensor_mul(t, xT_t[kt], scale_b)
                    nc.vector.tensor_add(xq8[g][:, i, :], t, bias_b)

                for j in range(NKT):
                    nc.vector.memset(qTz[j][0], 0.0)
                    nc.vector.memset(qTz[j][1], 0.0)

                # K (full seq) then Q (own half), feature-major, fp8 DoubleRow
                fix_i = 0
                qk_work = []
                for j in range(NKT):
                    for sc in range(4):
                        qk_work.append(("k", j, sc))
                    for sc in range(2):
                        qk_work.append(("q", j, sc))
                    for st in range(4 * j, 4 * j + 4):
                        qk_work.append(("v", j, st))
                for (kind, j, sc) in qk_work:
                    ps = ps_qkv.tile([P, FD], F32, name="ps_q", tag="qk")
                    if kind == "v":
                        st = sc
                        for g in range(2):
                            nc.tensor.matmul(
                                ps, xq8[g][:, :, st * P:(st + 1) * P],
                                Wv8_t[g], start=(g == 0), stop=(g == 1),
                                perf_mode=DR)
                        vv = vo8[:, st // 2, st % 2, :].rearrange(
                            "p (h c) -> p h c", c=DK + 2)
                        nc.scalar.mul(vv[:, :, 0:DK],
                                      ps.rearrange("p (h c) -> p h c", c=DK),
                                      1.0 / SCL)
                        nc.vector.memset(vv[:, :, DK:DK + 2], 1.0)
                        continue
                    Wt = Wk8_t if kind == "k" else Wq8_t
                    for g in range(2):
                        nc.tensor.matmul(
                            ps, Wt[g][:, :, j * P:(j + 1) * P],
                            xq8[g][:, :, sc * FD:(sc + 1) * FD],
                            start=(g == 0), stop=(g == 1),
                            perf_mode=DR)
                    if kind == "k":
                        dst = kT[j][:, sc * FD:(sc + 1) * FD]
                        if fix_i % 2 == 0:
                            nc.vector.tensor_scalar_mul(dst, ps, 1.0 / SCL)
                        else:
                            nc.scalar.mul(dst, ps, 1.0 / SCL)
                    else:
                        for h01 in range(2):
                            bp = 64 * h01
                            dst = qTz[j][h01][bp:bp + DK,
                                              sc * FD:(sc + 1) * FD]
                            if (fix_i + h01) % 2 == 0:
                                nc.vector.tensor_scalar_mul(
                                    dst, ps[bp:bp + DK, :], 1.0 / SCL)
                            else:
                                nc.scalar.mul(dst, ps[bp:bp + DK, :],
                                              1.0 / SCL)
                    fix_i += 1


            if KDBG:
                nc.sync.dma_start(d_kT, kT[0])
                nc.sync.dma_start(d_qT, qT[0])
                nc.sync.dma_start(d_vo8, vo8)
            # ========== phases 2+3: attention + FFN, per query-chunk ==========
            with ExitStack() as P2:
                pp = P2.enter_context(tc.tile_pool(name="pp", bufs=2))
                p2s = P2.enter_context(tc.tile_pool(name="p2s", bufs=2))
                p3 = P2.enter_context(tc.tile_pool(name="p3", bufs=1))
                p3s = P2.enter_context(tc.tile_pool(name="p3s", bufs=3))
                Wo8_t, W18_t, W2_t = [], [], []
                for g in range(2):
                    w = p3.tile([P, 2, D], F8, name=f"Wo8_{g}")
                    nc.gpsimd.dma_start(w, Wo8[g])
                    Wo8_t.append(w)
                for g in range(2):
                    w = p3.tile([P, 2, DFF], F8, name=f"W18_{g}")
                    nc.gpsimd.dma_start(w, W18[g])
                    W18_t.append(w)
                for m in range(NMT):
                    w = p3.tile([P, D], BF16, name=f"W2_{m}")
                    nc.gpsimd.dma_start(w, W2b[m * P:(m + 1) * P, :])
                    W2_t.append(w)
                ds1_sb = p3.tile([P, NMT], F32, name="ds1_sb")
                nc.sync.dma_start(ds1_sb, bass.AP(
                    tensor=ds1.tensor, offset=ds1.offset,
                    ap=[[1, P], [P, NMT]]))
                for sq in range(NSQT):
                    nc.gpsimd.dma_start(xre[sq], x_tok[sq * P:(sq + 1) * P, :])
                x2tok = [p3.tile([P, D], F32, name=f"x2t{sq}")
                         for sq in range(NSQT)]
                mv2 = p3.tile([P, 2, NSQT], F32, name="mv2")
                xn2_8 = [p3.tile([P, 2, SQ], F8, name=f"xn2_8_{g}")
                         for g in range(2)]
                oT_bf = [p3.tile([P, FD], BF16, name=f"oTb_{hp}")
                         for hp in range(4)]

                def attention(qc, ps_sc, ps_acc):
                    for hp in range(4):
                        accs = {}
                        for h01 in range(2):
                            accs[h01] = ps_acc.tile(
                                [DK + 2, FD], F32,
                                name=f"acc{h01}", tag=f"acc{h01}")
                        for g in range(8):
                            sgs = {}
                            for half in range(2):
                                kt = 2 * g + half
                                for h01 in range(2):
                                    sgc = ps_sc.tile([P, FD], F32, name="sg",
                                                     tag="sg", bufs=4)
                                    nc.tensor.matmul(
                                        sgc, kT[hp][:, kt * P:(kt + 1) * P],
                                        qTz[hp][h01][:,
                                                     qc * FD:(qc + 1) * FD])
                                    sgs[(half, h01)] = sgc
                            pgs = {}
                            for half in range(2):
                                for h01 in range(2):
                                    pgt = pp.tile([P, FD], BF16, name="pg",
                                                  tag="pg", bufs=8)
                                    if (half + h01) % 2 == 0:
                                        nc.scalar.activation(
                                            pgt, sgs[(half, h01)], AF.Exp)
                                    else:
                                        nc.vector.tensor_scalar(
                                            pgt.bitcast(I16),
                                            sgs[(half, h01)], A_EXP, B_EXP,
                                            op0=MUL, op1=ADD)
                                    pgs[(half, h01)] = pgt
                            for h01 in range(2):
                                h = 2 * hp + h01
                                for half in range(2):
                                    nc.tensor.matmul(
                                        accs[h01],
                                        vo8[:, g, half,
                                            h * (DK + 2):(h + 1) * (DK + 2)],
                                        pgs[(half, h01)],
                                        start=(g == 0 and half == 0),
                                        stop=(g == 7 and half == 1))
                        t = qc * 4 + hp
                        for h01 in range(2):
                            acc = accs[h01]
                            nc.vector.tensor_copy(
                                dnT[t][64 * h01:64 * h01 + 1, :],
                                acc[DK:DK + 1, :])
                            if h01 == 0:
                                nc.vector.tensor_copy(
                                    oT_bf[hp][0:DK, :], acc[0:DK, :])
                            else:
                                nc.scalar.copy(
                                    oT_bf[hp][DK:P, :], acc[0:DK, :])
                            nc.sync.dma_start(
                                scr_dn.ap()[2 * t + h01:2 * t + h01 + 1, :],
                                dnT[t][64 * h01:64 * h01 + 1, :])

                def normalize(qc):
                    rcp = p2s.tile([64, 64], F32R, name="rcp", tag="rcp")
                    nc.sync.dma_start(rcp, bass.AP(
                        tensor=scr_dn.ap().tensor, offset=qc * 8 * FD,
                        ap=[[64, 64], [1, 64]]))
                    with nc.allow_low_precision(reason="denom recip"):
                        nc.vector.reciprocal(rcp, rcp)
                    nc.vector.tensor_scalar_mul(rcp, rcp, OSC)
                    nc.sync.dma_start(bass.AP(
                        tensor=scr_dn2.ap().tensor, offset=qc * 8 * FD,
                        ap=[[64, 64], [1, 64]]), rcp)
                    for hp in range(4):
                        t = qc * 4 + hp
                        rb_sb = p2s.tile([P, FD], F32R, name="rbs", tag="rbs")
                        for par in range(2):
                            row = scr_dn2.ap()[2 * t + par:2 * t + par + 1, :]
                            nc.sync.dma_start(
                                rb_sb[64 * par:64 * par + 64, :],
                                bass.AP(tensor=row.tensor, offset=row.offset,
                                        ap=[[0, 64]] + row.ap[1:]))
                        g, i = hp // 2, hp % 2
                        nc.vector.tensor_mul(oT8[qc][g][:, i, :],
                                             oT_bf[hp], rb_sb)
                    if KDBG and qc == 0:
                        nc.sync.dma_start(d_oT8, oT8[0][0])

                def wo_stats(qc, psp):
                    # Wo token-major + residual + LN2 stats
                    for sl in range(4):
                        sq = qc * 4 + sl
                        ps = psp.tile([P, D], F32, name="ps_wo", tag="ffn")
                        for g in range(2):
                            nc.tensor.matmul(
                                ps, oT8[qc][g][:, :, sl * P:(sl + 1) * P],
                                Wo8_t[g], start=(g == 0), stop=(g == 1),
                                perf_mode=DR)
                        nc.vector.scalar_tensor_tensor(
                            x2tok[sq], ps, 1.0 / (SCL * OSC), xre[sq],
                            op0=MUL, op1=ADD)
                        st6b = p3s.tile([P, 6], F32, name="st6b", tag="st6b")
                        nc.vector.bn_stats(st6b, x2tok[sq])
                        nc.vector.bn_aggr(mv2[:, :, sq:sq + 1], st6b)
                        if KDBG and sq == 0:
                            nc.sync.dma_start(d_x2, x2tok[0])

                def ln2_ffn(qc, psp):
                    # LN2 scale/bias rows for this half
                    sc2 = p3s.tile([P, 4], F32, name="sc2", tag="ln2")
                    bi2 = p3s.tile([P, 4], F32, name="bi2", tag="ln2")
                    std2 = p3s.tile([P, 4], F32, name="std2", tag="ln2")
                    nc.scalar.activation(std2, mv2[:, 1, 4 * qc:4 * qc + 4],
                                         AF.Sqrt, bias=0.0,
                                         scale=float(D) / (D - 1))
                    nc.vector.tensor_scalar_add(std2, std2, EPS)
                    nc.vector.reciprocal(std2, std2)
                    nc.vector.tensor_scalar_mul(sc2, std2, float(g2))
                    nc.vector.tensor_mul(std2, mv2[:, 0, 4 * qc:4 * qc + 4], sc2)
                    nc.vector.tensor_scalar(bi2, std2, -1.0, float(be2),
                                            op0=MUL, op1=ADD)
                    s2b = p2s.tile([P, FD], F32, name="s2b", tag="s2b")
                    b2b = p2s.tile([P, FD], F32, name="b2b", tag="s2b")
                    for src, scr, dst in ((sc2, scr_s2, s2b),
                                          (bi2, scr_b2, b2b)):
                        tp2 = psp.tile([4, P], F32, name="tp2", tag="ffn")
                        nc.tensor.transpose(tp2, src, ident)
                        rw2 = p3s.tile([4, P], F32, name="rw2", tag="rw2")
                        nc.vector.tensor_copy(rw2, tp2)
                        nc.sync.dma_start(scr.ap()[4 * qc:4 * qc + 4, :], rw2)
                        nc.sync.dma_start(dst, bass.AP(
                            tensor=scr.ap().tensor,
                            offset=scr.ap().offset + qc * 4 * P,
                            ap=[[0, P], [1, FD]]))
                    if KDBG and qc == 0:
                        nc.sync.dma_start(d_s2b, s2b)
                        nc.sync.dma_start(d_b2b, b2b)
                    # Wo feature-major + residual in x^T layout + LN2 apply
                    for j in range(NKT):
                        ps = psp.tile([P, FD], F32, name="ps_woT", tag="ffn")
                        for g in range(2):
                            nc.tensor.matmul(ps, Wo8_t[g][:, :, j * P:(j + 1) * P],
                                             oT8[qc][g], start=(g == 0),
                                             stop=(g == 1), perf_mode=DR)
                        x2T = p3s.tile([P, FD], F32, name="x2T", tag="x2T",
                                       bufs=2)
                        nc.vector.scalar_tensor_tensor(
                            x2T, ps, 1.0 / (SCL * OSC),
                            xT_t[j][:, qc * FD:(qc + 1) * FD],
                            op0=MUL, op1=ADD)
                        if KDBG and qc == 0 and j == 0:
                            nc.sync.dma_start(d_x2T, x2T)
                        t2 = p3s.tile([P, FD], F32, name="t2", tag="t2",
                                      bufs=2)
                        eng = nc.gpsimd if (qc == 0 and j % 2 == 1) \
                            else nc.vector
                        eng.tensor_mul(t2, x2T, s2b)
                        g2i, i2 = j // 2, j % 2
                        eng.tensor_add(
                            xn2_8[g2i][:, i2, qc * FD:(qc + 1) * FD], t2, b2b)
                    if KDBG and qc == 1:
                        nc.sync.dma_start(d_xn2, xn2_8[0])
                    if KDBG and qc == 0:
                        nc.sync.dma_start(d_xn2b, xn2_8[0])
                    # FFN1 fp8 DoubleRow with per-column descale + relu
                    ffb = []
                    for mt in range(NMT):
                        ps = psp.tile([P, FD], F32, name="ps_f1", tag="ffn")
                        for g in range(2):
                            nc.tensor.matmul(
                                ps, W18_t[g][:, :, mt * P:(mt + 1) * P],
                                xn2_8[g][:, :, qc * FD:(qc + 1) * FD],
                                start=(g == 0), stop=(g == 1), perf_mode=DR)
                        ff = p3s.tile([P, FD], BF16, name=f"ff{mt}",
                                      tag=f"ff{mt}", bufs=1)
                        nc.scalar.activation(
                            ff, ps, AF.Relu, bias=0.0,
                            scale=ds1_sb[:, mt:mt + 1])
                        ffb.append(ff)
                        if KDBG and qc == 0 and mt == 0:
                            nc.sync.dma_start(d_ff, ff)
                    # FFN2 bf16
                    for sl in range(4):
                        sq = qc * 4 + sl
                        ps = psp.tile([P, D], F32, name="ps_f2", tag="ffn")
                        for mt in range(NMT):
                            nc.tensor.matmul(ps, ffb[mt][:, sl * P:(sl + 1) * P],
                                             W2_t[mt], start=(mt == 0),
                                             stop=(mt == NMT - 1))
                        ot = p3s.tile([P, D], F32, name="ot", tag="ot",
                                      bufs=2)
                        nc.vector.tensor_add(ot, ps, x2tok[sq])
                        nc.sync.dma_start(out[sq * P:(sq + 1) * P, :], ot)

                with ExitStack() as PA:
                    ps_sc = PA.enter_context(
                        tc.tile_pool(name="ps_sc", bufs=2, space="PSUM"))
                    ps_acc = PA.enter_context(
                        tc.tile_pool(name="ps_acc", bufs=1, space="PSUM"))
                    ps_big = PA.enter_context(
                        tc.tile_pool(name="ps_big", bufs=2, space="PSUM"))
                    attention(0, ps_sc, ps_acc)
                    normalize(0)
                    wo_stats(0, ps_big)
                    attention(1, ps_sc, ps_acc)
                    ln2_ffn(0, ps_big)
                    normalize(1)
                    wo_stats(1, ps_big)
                    ln2_ffn(1, ps_big)

    nc.compile()
    return nc


def _fast_path_ok(inputs):
    if not np.all(np.asarray(inputs["src_mask"]) != 0):
        return False
    for b in ("bq", "bk", "bv", "bo", "b1", "b2"):
        if np.any(np.asarray(inputs[b]) != 0):
            return False
    return True


def _pack_dr(W, scale):
    """Pack [K, M] weight into fp8 DoubleRow layout [K//256, 128, 2, M]."""
    import ml_dtypes
    K, M = W.shape
    Wp = (np.asarray(W, np.float32) * scale).reshape(K // 256, 2, P, M)
    Wp = Wp.transpose(0, 2, 1, 3)
    return np.ascontiguousarray(Wp.astype(ml_dtypes.float8_e4m3))


def kernel(**inputs):
    x = np.ascontiguousarray(np.asarray(inputs["x"], np.float32))
    g1 = float(np.asarray(inputs["g1"]))
    be1 = float(np.asarray(inputs["be1"]))
    g2 = float(np.asarray(inputs["g2"]))
    be2 = float(np.asarray(inputs["be2"]))

    if not _fast_path_ok(inputs):
        return _np_reference(**{k: np.asarray(v) for k, v in inputs.items()})

    from concourse.bass_utils import run_bass_kernel_spmd

    key = (g1, be1, g2, be2)
    if key not in _CACHE:
        _CACHE[key] = _build(*key)
    nc = _CACHE[key]

    import ml_dtypes
    BF = ml_dtypes.bfloat16
    scale = np.float32(1.0 / np.sqrt(DK))
    Wq8 = _pack_dr(inputs["Wq"], SCL * scale)
    Wk8 = _pack_dr(inputs["Wk"], SCL)
    Wv8 = _pack_dr(inputs["Wv"], SCL)
    Wo8 = _pack_dr(inputs["Wo"], SCL)
    W1 = np.asarray(inputs["W1"], np.float32)
    scl1 = 240.0 / np.abs(W1).max(0)
    W18 = _pack_dr(W1 * scl1, 1.0)
    ds1 = np.ascontiguousarray((1.0 / scl1).reshape(NMT, P).astype(np.float32))
    W2b = np.ascontiguousarray(np.asarray(inputs["W2"], np.float32).astype(BF))

    in_maps = []
    for c in range(8):
        b, hh = c // 2, c % 2
        if hh == 0:
            xp = x[b]
        else:
            xp = np.concatenate([x[b, SQ:], x[b, :SQ]], axis=0)
        xp = np.ascontiguousarray(xp)
        in_maps.append(dict(
            xT=np.ascontiguousarray(xp.T.astype(BF)),
            x_tok=np.ascontiguousarray(xp[:SQ]),
            Wq8=Wq8, Wk8=Wk8, Wv8=Wv8, Wo8=Wo8, W18=W18, ds1=ds1, W2b=W2b))

    res = run_bass_kernel_spmd(nc, in_maps, core_ids=list(range(8)),
                               trace=_TRACE["trace"],
                               trace_cores=_TRACE["trace_cores"])
    _LAST["res"] = res

    full = np.empty((B, S, D), np.float32)
    for c in range(8):
        b, hh = c // 2, c % 2
        full[b, hh * SQ:(hh + 1) * SQ] = res.results[c]["out"]
    return full


# revision 36
# speedup vs baseline: 1.2001x; 1.0157x over previous
"""Trainium2 Bass kernel for a pre-LN transformer encoder block.

Problem: x[4, 2048, 512], H=8 heads, d_ff=2048, f32.
Sharding: 8 cores = (batch b, seq-half). Each core computes the block for
1024 query rows of batch b; K/V cover the full 2048-row sequence of that
batch (duplicated across the pair) so no collectives are needed. The host
permutes each core's sequence so its own 1024 queries come first.

On-core dataflow:
  LN1 stats feature-major via PE column-sum matmuls (ones lhsT) over xT and
  xT^2; stats math on a [16,128] layout; scale/bias rows broadcast from DRAM.
  xn materialized once in fp8e4 DoubleRow layout [128, 2, S]; Q/K/V/Wo/FFN1
  run as fp8 DoubleRow matmuls (2x PE throughput). Scores stay bf16
  (contraction is only 64). Softmax exp is split across the Scalar engine
  (exact Exp with fp8 output) and the Vector engine (Schraudolph bit-trick:
  fp8 bits = s*8*log2e + 56 written through an int8 bitcast), alternating
  per key-block so neither engine is the bottleneck. attn@V runs fp8
  DoubleRow with a ones-column in V so the softmax denominator falls out of
  accumulator row 64; normalization multiplies by the broadcast reciprocal
  (x64 so o fits fp8). Wo runs twice (token- and feature-major, both fp8).
  FFN1 fp8 with per-column absmax weight quant (descale folded into relu);
  FFN2 bf16 for accuracy; final residual + store.
"""

import sys
import numpy as np

sys.path.insert(0, "/opt/trn_rl_repo")

B, S, D = 4, 2048, 512
H, DK, DFF = 8, 64, 2048
SQ = S // 2
P = 128
FD = 512
EPS = 1e-6
NKT = D // P          # 4  feature tiles
NST = S // P          # 16 sequence tiles
NSQT = SQ // P        # 8  own-query tiles
NMT = DFF // P        # 16 ffn tiles
SCL = 128.0           # fp8 weight prescale
OSC = 64.0            # fp8 o prescale
A_EXP = float(128.0 / np.log(2.0))
B_EXP = 16249.6       # 127*128 - 6.9 schraudolph shift (bf16 bits)

_CACHE = {}
_TRACE = {"trace": False, "trace_cores": None}
_LAST = {"res": None}


def _np_reference(x, src_mask, Wq, bq, Wk, bk, Wv, bv, Wo, bo,
                  W1, b1, W2, b2, g1, be1, g2, be2):
    """Faithful numpy fallback (used only for off-nominal inputs)."""
    x = np.asarray(x, np.float32)

    def ln(t, g, be):
        m = t.mean(-1, keepdims=True)
        var = ((t - m) ** 2).sum(-1, keepdims=True) / (t.shape[-1] - 1)
        return g * (t - m) / (np.sqrt(var) + EPS) + be

    Bv, Sv, _ = x.shape
    xn = ln(x, g1, be1)
    q = (xn @ Wq + bq).reshape(Bv, Sv, H, DK).transpose(0, 2, 1, 3)
    k = (xn @ Wk + bk).reshape(Bv, Sv, H, DK).transpose(0, 2, 1, 3)
    v = (xn @ Wv + bv).reshape(Bv, Sv, H, DK).transpose(0, 2, 1, 3)
    s = np.einsum("bhqd,bhkd->bhqk", q, k) / np.float32(np.sqrt(DK))
    s = np.where(np.asarray(src_mask) == 0, np.float32(-1e9), s)
    s = s - s.max(-1, keepdims=True)
    p = np.exp(s)
    p = p / p.sum(-1, keepdims=True)
    o = np.einsum("bhqk,bhkd->bhqd", p, v)
    o = o.transpose(0, 2, 1, 3).reshape(Bv, Sv, D)
    x = x + o @ Wo + bo
    xn = ln(x, g2, be2)
    return (x + np.maximum(xn @ W1 + b1, 0.0) @ W2 + b2).astype(np.float32)


def _build(g1, be1, g2, be2):
    import concourse.bass as bass
    import concourse.tile as tile
    from concourse import bacc, mybir
    from concourse.masks import make_identity
    from contextlib import ExitStack

    F32 = mybir.dt.float32
    F32R = mybir.dt.float32r
    BF16 = mybir.dt.bfloat16
    F8 = mybir.dt.float8e4
    I16 = mybir.dt.int16
    AF = mybir.ActivationFunctionType
    MUL = mybir.AluOpType.mult
    ADD = mybir.AluOpType.add
    MAX = mybir.AluOpType.max
    DR = mybir.MatmulPerfMode.DoubleRow

    nc = bacc.Bacc("TRN2", target_bir_lowering=False, debug=False)

    xT = nc.dram_tensor("xT", [D, S], BF16, kind="ExternalInput").ap()
    x_tok = nc.dram_tensor("x_tok", [SQ, D], F32, kind="ExternalInput").ap()
    Wq8 = nc.dram_tensor("Wq8", [2, P, 2, D], F8, kind="ExternalInput").ap()
    Wk8 = nc.dram_tensor("Wk8", [2, P, 2, D], F8, kind="ExternalInput").ap()
    Wv8 = nc.dram_tensor("Wv8", [2, P, 2, D], F8, kind="ExternalInput").ap()
    Wo8 = nc.dram_tensor("Wo8", [2, P, 2, D], F8, kind="ExternalInput").ap()
    W18 = nc.dram_tensor("W18", [2, P, 2, DFF], F8, kind="ExternalInput").ap()
    ds1 = nc.dram_tensor("ds1", [NMT, P], F32, kind="ExternalInput").ap()
    W2b = nc.dram_tensor("W2b", [DFF, D], BF16, kind="ExternalInput").ap()

    out = nc.dram_tensor("out", [SQ, D], F32, kind="ExternalOutput").ap()
    scr_st = nc.dram_tensor("scr_st", [2, S], F32)
    scr_r1 = nc.dram_tensor("scr_r1", [2, S], BF16)
    import os
    KDBG = int(os.environ.get("KDBG", "0"))
    if KDBG:
        d_kT = nc.dram_tensor("d_kT", [P, S], BF16, kind="ExternalOutput").ap()
        d_qT = nc.dram_tensor("d_qT", [P, SQ], BF16, kind="ExternalOutput").ap()
        d_vo8 = nc.dram_tensor("d_vo8", [P, 8, 2, H * (DK + 2)], F8,
                               kind="ExternalOutput").ap()
        d_pg = nc.dram_tensor("d_pg", [P, 8, 2 * FD], F8,
                              kind="ExternalOutput").ap()
        d_oT8 = nc.dram_tensor("d_oT8", [P, 2, FD], F8,
                               kind="ExternalOutput").ap()
        d_x2 = nc.dram_tensor("d_x2", [P, D], F32, kind="ExternalOutput").ap()
        d_xn2 = nc.dram_tensor("d_xn2", [P, 2, SQ], F8,
                               kind="ExternalOutput").ap()
        d_scb = nc.dram_tensor("d_scb", [P, S], BF16,
                               kind="ExternalOutput").ap()
        d_bib = nc.dram_tensor("d_bib", [P, S], BF16,
                               kind="ExternalOutput").ap()
        d_ff = nc.dram_tensor("d_ff", [P, FD], BF16,
                              kind="ExternalOutput").ap()
        d_s2b = nc.dram_tensor("d_s2b", [P, FD], F32,
                               kind="ExternalOutput").ap()
        d_b2b = nc.dram_tensor("d_b2b", [P, FD], F32,
                               kind="ExternalOutput").ap()
        d_x2T = nc.dram_tensor("d_x2T", [P, FD], F32,
                               kind="ExternalOutput").ap()
        d_rw2 = nc.dram_tensor("d_rw2", [4, P], BF16,
                               kind="ExternalOutput").ap()
        d_xn2b = nc.dram_tensor("d_xn2b", [P, 2, SQ], F8,
                                kind="ExternalOutput").ap()

    scr_s2 = nc.dram_tensor("scr_s2", [NSQT, P], F32)
    scr_b2 = nc.dram_tensor("scr_b2", [NSQT, P], F32)
    scr_dn = nc.dram_tensor("scr_dn", [16, FD], F32R)
    scr_dn2 = nc.dram_tensor("scr_dn2", [16, FD], F32R)

    def bcast_row(src_dram, nfree):
        return bass.AP(tensor=src_dram.tensor, offset=src_dram.offset,
                       ap=[[0, P], [1, nfree]])

    with tile.TileContext(nc) as tc, ExitStack() as OU:
        res = OU.enter_context(tc.tile_pool(name="res", bufs=1))

        ident = res.tile([P, P], F32, name="ident")
        make_identity(nc, ident)
        dnT = [res.tile([65, FD], F32R, name=f"dnT{t}") for t in range(8)]
        # fp8 o in DoubleRow layout: oT8[qc][g][:, i, :] holds o-features
        # g*256+i*128+p for query chunk qc
        oT8 = [[res.tile([P, 2, FD], F8, name=f"oT8_{qc}_{g}") for g in range(2)]
               for qc in range(2)]
        xT_t = [res.tile([P, S], BF16, name=f"xT{j}") for j in range(NKT)]
        xre = [res.tile([P, D], F32, name=f"xre{sq}") for sq in range(NSQT)]

        with ExitStack() as QK:
            qkv = QK.enter_context(tc.tile_pool(name="qkv", bufs=1))
            qTz = [[qkv.tile([P, SQ], BF16, name=f"qTz{j}_{h01}")
                    for h01 in range(2)] for j in range(NKT)]
            kT = [qkv.tile([P, S], BF16, name=f"kT{j}") for j in range(NKT)]
            vo8 = qkv.tile([P, 8, 2, H * (DK + 2)], BF16, name="vo8")

            # ================= phase 1: LN1 stats + QKV =================
            with ExitStack() as P1:
                p1 = P1.enter_context(tc.tile_pool(name="p1", bufs=1))
                p1s = P1.enter_context(tc.tile_pool(name="p1s", bufs=3))
                ps_qkv = P1.enter_context(
                    tc.tile_pool(name="ps_qkv", bufs=6, space="PSUM"))

                for ch in range(2):
                    for j in range(NKT):
                        eng = nc.sync if j % 2 == 0 else nc.scalar
                        eng.dma_start(
                            xT_t[j][:, ch * SQ:(ch + 1) * SQ],
                            xT[j * P:(j + 1) * P, ch * SQ:(ch + 1) * SQ])
                Wq8_t, Wk8_t, Wv8_t = [], [], []
                for g in range(2):
                    w = p1.tile([P, 2, D], F8, name=f"Wk8_{g}")
                    nc.gpsimd.dma_start(w, Wk8[g])
                    Wk8_t.append(w)
                for g in range(2):
                    w = p1.tile([P, 2, D], F8, name=f"Wq8_{g}")
                    nc.gpsimd.dma_start(w, Wq8[g])
                    Wq8_t.append(w)
                for g in range(2):
                    w = p1.tile([P, 2, D], F8, name=f"Wv8_{g}")
                    nc.gpsimd.dma_start(w, Wv8[g])
                    Wv8_t.append(w)

                ones_bf = p1.tile([P, P], BF16, name="ones_bf")
                nc.vector.memset(ones_bf, 1.0)

                # colsum stats: sum x and sum x^2 over features, per token
                xsq = [p1.tile([P, S], BF16, name=f"xsq{j}") for j in range(NKT)]
                for j in range(NKT):
                    if j % 2 == 0:
                        nc.vector.tensor_mul(xsq[j], xT_t[j], xT_t[j])
                    else:
                        nc.scalar.square(xsq[j], xT_t[j])
                st_sa = p1.tile([1, S], F32, name="st_sa")
                st_sq = p1.tile([1, S], F32, name="st_sq")
                for c in range(4):
                    ps1 = ps_qkv.tile([P, FD], F32, name=f"s1_{c}", tag="qk")
                    ps2 = ps_qkv.tile([P, FD], F32, name=f"s2_{c}", tag="qk")
                    for j in range(NKT):
                        nc.tensor.matmul(ps1, ones_bf,
                                         xT_t[j][:, c * FD:(c + 1) * FD],
                                         start=(j == 0), stop=(j == NKT - 1))
                    for j in range(NKT):
                        nc.tensor.matmul(ps2, ones_bf,
                                         xsq[j][:, c * FD:(c + 1) * FD],
                                         start=(j == 0), stop=(j == NKT - 1))
                    nc.vector.tensor_copy(st_sa[0:1, c * FD:(c + 1) * FD],
                                          ps1[0:1, :])
                    nc.scalar.copy(st_sq[0:1, c * FD:(c + 1) * FD],
                                   ps2[0:1, :])
                nc.sync.dma_start(scr_st.ap()[0:1, :], st_sa)
                nc.sync.dma_start(scr_st.ap()[1:2, :], st_sq)

                def row16(scr, row):
                    return bass.AP(tensor=scr.ap().tensor,
                                   offset=scr.ap().offset + row * S,
                                   ap=[[P, 16], [1, P]])

                s16a = p1.tile([16, P], F32, name="s16a")
                s16b = p1.tile([16, P], F32, name="s16b")
                nc.sync.dma_start(s16a, row16(scr_st, 0))
                nc.sync.dma_start(s16b, row16(scr_st, 1))
                m16 = p1.tile([16, P], F32, name="m16")
                t16 = p1.tile([16, P], F32, name="t16")
                sd16 = p1.tile([16, P], F32, name="sd16")
                sc16 = p1.tile([16, P], BF16, name="sc16")
                bi16 = p1.tile([16, P], BF16, name="bi16")
                nc.vector.tensor_scalar_mul(m16, s16a, 1.0 / D)
                nc.vector.tensor_mul(t16, m16, s16a)
                nc.vector.tensor_sub(t16, s16b, t16)
                nc.scalar.activation(sd16, t16, AF.Sqrt, bias=0.0,
                                     scale=1.0 / (D - 1))
                nc.vector.tensor_scalar_add(sd16, sd16, EPS)
                nc.vector.reciprocal(sd16, sd16)
                nc.vector.tensor_scalar_mul(sc16, sd16, float(g1))
                nc.vector.tensor_mul(t16, m16, sc16)
                nc.vector.tensor_scalar(bi16, t16, -1.0, float(be1),
                                        op0=MUL, op1=ADD)

                def row16w(scr, row):
                    return bass.AP(tensor=scr.ap().tensor,
                                   offset=scr.ap().offset + row * S,
                                   ap=[[P, 16], [1, P]])

                nc.sync.dma_start(row16w(scr_r1, 0), sc16)
                nc.sync.dma_start(row16w(scr_r1, 1), bi16)
                scale_b = p1.tile([P, S], BF16, name="scale_b")
                bias_b = p1.tile([P, S], BF16, name="bias_b")
                nc.sync.dma_start(scale_b, bcast_row(scr_r1.ap()[0:1, :], S))
                nc.sync.dma_start(bias_b, bcast_row(scr_r1.ap()[1:2, :], S))

                # materialize xn in fp8 DoubleRow layout
                xq8 = [p1.tile([P, 2, S], F8, name=f"xq8_{g}") for g in range(2)]
                for kt in range(NKT):
                    g, i = kt // 2, kt % 2
                    t = p1s.tile([P, S], BF16, name="xnt", tag=f"xnt{kt % 2}")
                    nc.vector.tensor_mul(t, xT_t[kt], scale_b)
                    nc.vector.tensor_add(xq8[g][:, i, :], t, bias_b)

                for j in range(NKT):
                    nc.vector.memset(qTz[j][0], 0.0)
                    nc.vector.memset(qTz[j][1], 0.0)

                # K (full seq) then Q (own half), feature-major, fp8 DoubleRow
                fix_i = 0
                qk_work = []
                for j in range(NKT):
                    for sc in range(4):
                        qk_work.append(("k", j, sc))
                    for sc in range(2):
                        qk_work.append(("q", j, sc))
                    for st in range(4 * j, 4 * j + 4):
                        qk_work.append(("v", j, st))
                for (kind, j, sc) in qk_work:
                    ps = ps_qkv.tile([P, FD], F32, name="ps_q", tag="qk")
                    if kind == "v":
                        st = sc
                        for g in range(2):
                            nc.tensor.matmul(
                                ps, xq8[g][:, :, st * P:(st + 1) * P],
                                Wv8_t[g], start=(g == 0), stop=(g == 1),
                                perf_mode=DR)
                        vv = vo8[:, st // 2, st % 2, :].rearrange(
                            "p (h c) -> p h c", c=DK + 2)
                        nc.scalar.mul(vv[:, :, 0:DK],
                                      ps.rearrange("p (h c) -> p h c", c=DK),
                                      1.0 / SCL)
                        nc.vector.memset(vv[:, :, DK:DK + 2], 1.0)
                        continue
                    Wt = Wk8_t if kind == "k" else Wq8_t
                    for g in range(2):
                        nc.tensor.matmul(
                            ps, Wt[g][:, :, j * P:(j + 1) * P],
                            xq8[g][:, :, sc * FD:(sc + 1) * FD],
                            start=(g == 0), stop=(g == 1),
                            perf_mode=DR)
                    if kind == "k":
                        dst = kT[j][:, sc * FD:(sc + 1) * FD]
                        if fix_i % 2 == 0:
                            nc.vector.tensor_scalar_mul(dst, ps, 1.0 / SCL)
                        else:
                            nc.scalar.mul(dst, ps, 1.0 / SCL)
                    else:
                        for h01 in range(2):
                            bp = 64 * h01
                            dst = qTz[j][h01][bp:bp + DK,
                                              sc * FD:(sc + 1) * FD]
                            if (fix_i + h01) % 2 == 0:
                                nc.vector.tensor_scalar_mul(
                                    dst, ps[bp:bp + DK, :], 1.0 / SCL)
                            else:
                                nc.scalar.mul(dst, ps[bp:bp + DK, :],
                                              1.0 / SCL)
                    fix_i += 1


            if KDBG:
                nc.sync.dma_start(d_kT, kT[0])
                nc.sync.dma_start(d_qT, qT[0])
                nc.sync.dma_start(d_vo8, vo8)
            # ========== phases 2+3: attention + FFN, per query-chunk ==========
            with ExitStack() as P2:
                pp = P2.enter_context(tc.tile_pool(name="pp", bufs=2))
                p2s = P2.enter_context(tc.tile_pool(name="p2s", bufs=2))
                p3 = P2.enter_context(tc.tile_pool(name="p3", bufs=1))
                p3s = P2.enter_context(tc.tile_pool(name="p3s", bufs=3))
                Wo8_t, W18_t, W2_t = [], [], []
                for g in range(2):
                    w = p3.tile([P, 2, D], F8, name=f"Wo8_{g}")
                    nc.gpsimd.dma_start(w, Wo8[g])
                    Wo8_t.append(w)
                for g in range(2):
                    w = p3.tile([P, 2, DFF], F8, name=f"W18_{g}")
                    nc.gpsimd.dma_start(w, W18[g])
                    W18_t.append(w)
                for m in range(NMT):
                    w = p3.tile([P, D], BF16, name=f"W2_{m}")
                    nc.gpsimd.dma_start(w, W2b[m * P:(m + 1) * P, :])
                    W2_t.append(w)
                ds1_sb = p3.tile([P, NMT], F32, name="ds1_sb")
                nc.sync.dma_start(ds1_sb, bass.AP(
                    tensor=ds1.tensor, offset=ds1.offset,
                    ap=[[1, P], [P, NMT]]))
                for sq in range(NSQT):
                    nc.gpsimd.dma_start(xre[sq], x_tok[sq * P:(sq + 1) * P, :])
                x2tok = [p3.tile([P, D], F32, name=f"x2t{sq}")
                         for sq in range(NSQT)]
                mv2 = p3.tile([P, 2, NSQT], F32, name="mv2")
                xn2_8 = [p3.tile([P, 2, SQ], F8, name=f"xn2_8_{g}")
                         for g in range(2)]
                oT_bf = [p3.tile([P, FD], BF16, name=f"oTb_{hp}")
                         for hp in range(4)]

                def attention(qc, ps_sc, ps_acc):
                    for hp in range(4):
                        accs = {}
                        for h01 in range(2):
                            accs[h01] = ps_acc.tile(
                                [DK + 2, FD], F32,
                                name=f"acc{h01}", tag=f"acc{h01}")
                        for g in range(8):
                            sgs = {}
                            for half in range(2):
                                kt = 2 * g + half
                                for h01 in range(2):
                                    sgc = ps_sc.tile([P, FD], F32, name="sg",
                                                     tag="sg", bufs=4)
                                    nc.tensor.matmul(
                                        sgc, kT[hp][:, kt * P:(kt + 1) * P],
                                        qTz[hp][h01][:,
                                                     qc * FD:(qc + 1) * FD])
                                    sgs[(half, h01)] = sgc
                            pgs = {}
                            for half in range(2):
                                for h01 in range(2):
                                    pgt = pp.tile([P, FD], BF16, name="pg",
                                                  tag="pg", bufs=8)
                                    if (half + h01) % 2 == 0:
                                        nc.scalar.activation(
                                            pgt, sgs[(half, h01)], AF.Exp)
                                    else:
                                        nc.vector.tensor_scalar(
                                            pgt.bitcast(I16),
                                            sgs[(half, h01)], A_EXP, B_EXP,
                                            op0=MUL, op1=ADD)
                                    pgs[(half, h01)] = pgt
                            for h01 in range(2):
                                h = 2 * hp + h01
                                for half in range(2):
                                    nc.tensor.matmul(
                                        accs[h01],
                                        vo8[:, g, half,
                                            h * (DK + 2):(h + 1) * (DK + 2)],
                                        pgs[(half, h01)],
                                        start=(g == 0 and half == 0),
                                        stop=(g == 7 and half == 1))
                        t = qc * 4 + hp
                        for h01 in range(2):
                            acc = accs[h01]
                            nc.scalar.copy(
                                dnT[t][64 * h01:64 * h01 + 1, :],
                                acc[DK:DK + 1, :])
                            if h01 == 0:
                                nc.scalar.copy(
                                    oT_bf[hp][0:DK, :], acc[0:DK, :])
                            else:
                                nc.scalar.copy(
                                    oT_bf[hp][DK:P, :], acc[0:DK, :])
                            nc.sync.dma_start(
                                scr_dn.ap()[2 * t + h01:2 * t + h01 + 1, :],
                                dnT[t][64 * h01:64 * h01 + 1, :])

                def normalize(qc):
                    rcp = p2s.tile([64, 64], F32R, name="rcp", tag="rcp")
                    nc.sync.dma_start(rcp, bass.AP(
                        tensor=scr_dn.ap().tensor, offset=qc * 8 * FD,
                        ap=[[64, 64], [1, 64]]))
                    with nc.allow_low_precision(reason="denom recip"):
                        nc.vector.reciprocal(rcp, rcp)
                    nc.vector.tensor_scalar_mul(rcp, rcp, OSC)
                    nc.sync.dma_start(bass.AP(
                        tensor=scr_dn2.ap().tensor, offset=qc * 8 * FD,
                        ap=[[64, 64], [1, 64]]), rcp)
                    for hp in range(4):
                        t = qc * 4 + hp
                        rb_sb = p2s.tile([P, FD], F32R, name="rbs", tag="rbs")
                        for par in range(2):
                            row = scr_dn2.ap()[2 * t + par:2 * t + par + 1, :]
                            nc.sync.dma_start(
                                rb_sb[64 * par:64 * par + 64, :],
                                bass.AP(tensor=row.tensor, offset=row.offset,
                                        ap=[[0, 64]] + row.ap[1:]))
                        g, i = hp // 2, hp % 2
                        nc.vector.tensor_mul(oT8[qc][g][:, i, :],
                                             oT_bf[hp], rb_sb)
                    if KDBG and qc == 0:
                        nc.sync.dma_start(d_oT8, oT8[0][0])

                def wo_stats(qc, psp):
                    # Wo token-major + residual + LN2 stats
                    for sl in range(4):
                        sq = qc * 4 + sl
                        ps = psp.tile([P, D], F32, name="ps_wo", tag="ffn")
                        for g in range(2):
                            nc.tensor.matmul(
                                ps, oT8[qc][g][:, :, sl * P:(sl + 1) * P],
                                Wo8_t[g], start=(g == 0), stop=(g == 1),
                                perf_mode=DR)
                        nc.vector.scalar_tensor_tensor(
                            x2tok[sq], ps, 1.0 / (SCL * OSC), xre[sq],
                            op0=MUL, op1=ADD)
                        st6b = p3s.tile([P, 6], F32, name="st6b", tag="st6b")
                        nc.vector.bn_stats(st6b, x2tok[sq])
                        nc.vector.bn_aggr(mv2[:, :, sq:sq + 1], st6b)
                        if KDBG and sq == 0:
                            nc.sync.dma_start(d_x2, x2tok[0])

                def ln2_ffn(qc, psp):
                    # LN2 scale/bias rows for this half
                    sc2 = p3s.tile([P, 4], F32, name="sc2", tag="ln2")
                    bi2 = p3s.tile([P, 4], F32, name="bi2", tag="ln2")
                    std2 = p3s.tile([P, 4], F32, name="std2", tag="ln2")
                    nc.scalar.activation(std2, mv2[:, 1, 4 * qc:4 * qc + 4],
                                         AF.Sqrt, bias=0.0,
                                         scale=float(D) / (D - 1))
                    nc.vector.tensor_scalar_add(std2, std2, EPS)
                    nc.vector.reciprocal(std2, std2)
                    nc.vector.tensor_scalar_mul(sc2, std2, float(g2))
                    nc.vector.tensor_mul(std2, mv2[:, 0, 4 * qc:4 * qc + 4], sc2)
                    nc.vector.tensor_scalar(bi2, std2, -1.0, float(be2),
                                            op0=MUL, op1=ADD)
                    s2b = p2s.tile([P, FD], F32, name="s2b", tag="s2b")
                    b2b = p2s.tile([P, FD], F32, name="b2b", tag="s2b")
                    for src, scr, dst in ((sc2, scr_s2, s2b),
                                          (bi2, scr_b2, b2b)):
                        tp2 = psp.tile([4, P], F32, name="tp2", tag="ffn")
                        nc.tensor.transpose(tp2, src, ident)
                        rw2 = p3s.tile([4, P], F32, name="rw2", tag="rw2")
                        nc.vector.tensor_copy(rw2, tp2)
                        nc.sync.dma_start(scr.ap()[4 * qc:4 * qc + 4, :], rw2)
                        nc.sync.dma_start(dst, bass.AP(
                            tensor=scr.ap().tensor,
                            offset=scr.ap().offset + qc * 4 * P,
                            ap=[[0, P], [1, FD]]))
                    if KDBG and qc == 0:
                        nc.sync.dma_start(d_s2b, s2b)
                        nc.sync.dma_start(d_b2b, b2b)
                    # Wo feature-major + residual in x^T layout + LN2 apply
                    for j in range(NKT):
                        ps = psp.tile([P, FD], F32, name="ps_woT", tag="ffn")
                        for g in range(2):
                            nc.tensor.matmul(ps, Wo8_t[g][:, :, j * P:(j + 1) * P],
                                             oT8[qc][g], start=(g == 0),
                                             stop=(g == 1), perf_mode=DR)
                        x2T = p3s.tile([P, FD], F32, name="x2T", tag="x2T",
                                       bufs=2)
                        nc.vector.scalar_tensor_tensor(
                            x2T, ps, 1.0 / (SCL * OSC),
                            xT_t[j][:, qc * FD:(qc + 1) * FD],
                            op0=MUL, op1=ADD)
                        if KDBG and qc == 0 and j == 0:
                            nc.sync.dma_start(d_x2T, x2T)
                        t2 = p3s.tile([P, FD], F32, name="t2", tag="t2",
                                      bufs=2)
                        eng = nc.gpsimd if (qc == 0 and j % 2 == 1) \
                            else nc.vector
                        eng.tensor_mul(t2, x2T, s2b)
                        g2i, i2 = j // 2, j % 2
                        eng.tensor_add(
                            xn2_8[g2i][:, i2, qc * FD:(qc + 1) * FD], t2, b2b)
                    if KDBG and qc == 1:
                        nc.sync.dma_start(d_xn2, xn2_8[0])
                    if KDBG and qc == 0:
                        nc.sync.dma_start(d_xn2b, xn2_8[0])
                    # FFN1 fp8 DoubleRow with per-column descale + relu
                    ffb = []
                    for mt in range(NMT):
                        ps = psp.tile([P, FD], F32, name="ps_f1", tag="ffn")
                        for g in range(2):
                            nc.tensor.matmul(
                                ps, W18_t[g][:, :, mt * P:(mt + 1) * P],
                                xn2_8[g][:, :, qc * FD:(qc + 1) * FD],
                                start=(g == 0), stop=(g == 1), perf_mode=DR)
                        ff = p3s.tile([P, FD], BF16, name=f"ff{mt}",
                                      tag=f"ff{mt}", bufs=1)
                        nc.scalar.activation(
                            ff, ps, AF.Relu, bias=0.0,
                            scale=ds1_sb[:, mt:mt + 1])
                        ffb.append(ff)
                        if KDBG and qc == 0 and mt == 0:
                            nc.sync.dma_start(d_ff, ff)
                    # FFN2 bf16
                    for sl in range(4):
                        sq = qc * 4 + sl
                        ps = psp.tile([P, D], F32, name="ps_f2", tag="ffn")
                        for mt in range(NMT):
                            nc.tensor.matmul(ps, ffb[mt][:, sl * P:(sl + 1) * P],
                                             W2_t[mt], start=(mt == 0),
                                             stop=(mt == NMT - 1))
                        ot = p3s.tile([P, D], F32, name="ot", tag="ot",
                                      bufs=2)
                        nc.vector.tensor_add(ot, ps, x2tok[sq])
                        nc.sync.dma_start(out[sq * P:(sq + 1) * P, :], ot)

                with ExitStack() as PA:
                    ps_sc = PA.enter_context(
                        tc.tile_pool(name="ps_sc", bufs=2, space="PSUM"))
                    ps_acc = PA.enter_context(
                        tc.tile_pool(name="ps_acc", bufs=1, space="PSUM"))
                    ps_big = PA.enter_context(
                        tc.tile_pool(name="ps_big", bufs=2, space="PSUM"))
                    attention(0, ps_sc, ps_acc)
                    normalize(0)
                    wo_stats(0, ps_big)
                    attention(1, ps_sc, ps_acc)
                    ln2_ffn(0, ps_big)
                    normalize(1)
                    wo_stats(1, ps_big)
                    ln2_ffn(1, ps_big)

    nc.compile()
    return nc


def _fast_path_ok(inputs):
    if not np.all(np.asarray(inputs["src_mask"]) != 0):
        return False
    for b in ("bq", "bk", "bv", "bo", "b1", "b2"):
        if np.any(np.asarray(inputs[b]) != 0):
            return False
    return True


def _pack_dr(W, scale):
    """Pack [K, M] weight into fp8 DoubleRow layout [K//256, 128, 2, M]."""
    import ml_dtypes
    K, M = W.shape
    Wp = (np.asarray(W, np.float32) * scale).reshape(K // 256, 2, P, M)
    Wp = Wp.transpose(0, 2, 1, 3)
    return np.ascontiguousarray(Wp.astype(ml_dtypes.float8_e4m3))


def kernel(**inputs):
    x = np.ascontiguousarray(np.asarray(inputs["x"], np.float32))
    g1 = float(np.asarray(inputs["g1"]))
    be1 = float(np.asarray(inputs["be1"]))
    g2 = float(np.asarray(inputs["g2"]))
    be2 = float(np.asarray(inputs["be2"]))

    if not _fast_path_ok(inputs):
        return _np_reference(**{k: np.asarray(v) for k, v in inputs.items()})

    from concourse.bass_utils import run_bass_kernel_spmd

    key = (g1, be1, g2, be2)
    if key not in _CACHE:
        _CACHE[key] = _build(*key)
    nc = _CACHE[key]

    import ml_dtypes
    BF = ml_dtypes.bfloat16
    scale = np.float32(1.0 / np.sqrt(DK))
    Wq8 = _pack_dr(inputs["Wq"], SCL * scale)
    Wk8 = _pack_dr(inputs["Wk"], SCL)
    Wv8 = _pack_dr(inputs["Wv"], SCL)
    Wo8 = _pack_dr(inputs["Wo"], SCL)
    W1 = np.asarray(inputs["W1"], np.float32)
    scl1 = 240.0 / np.abs(W1).max(0)
    W18 = _pack_dr(W1 * scl1, 1.0)
    ds1 = np.ascontiguousarray((1.0 / scl1).reshape(NMT, P).astype(np.float32))
    W2b = np.ascontiguousarray(np.asarray(inputs["W2"], np.float32).astype(BF))

    in_maps = []
    for c in range(8):
        b, hh = c // 2, c % 2
        if hh == 0:
            xp = x[b]
        else:
            xp = np.concatenate([x[b, SQ:], x[b, :SQ]], axis=0)
        xp = np.ascontiguousarray(xp)
        in_maps.append(dict(
            xT=np.ascontiguousarray(xp.T.astype(BF)),
            x_tok=np.ascontiguousarray(xp[:SQ]),
            Wq8=Wq8, Wk8=Wk8, Wv8=Wv8, Wo8=Wo8, W18=W18, ds1=ds1, W2b=W2b))

    res = run_bass_kernel_spmd(nc, in_maps, core_ids=list(range(8)),
                               trace=_TRACE["trace"],
                               trace_cores=_TRACE["trace_cores"])
    _LAST["res"] = res

    full = np.empty((B, S, D), np.float32)
    for c in range(8):
        b, hh = c // 2, c % 2
        full[b, hh * SQ:(hh + 1) * SQ] = res.results[c]["out"]
    return full


# revision 37
# speedup vs baseline: 1.2578x; 1.0480x over previous
"""Trainium2 Bass kernel for a pre-LN transformer encoder block.

Problem: x[4, 2048, 512], H=8 heads, d_ff=2048, f32.
Sharding: 8 cores = (batch b, seq-half). Each core computes the block for
1024 query rows of batch b; K/V cover the full 2048-row sequence of that
batch (duplicated across the pair) so no collectives are needed. The host
permutes each core's sequence so its own 1024 queries come first.

On-core dataflow:
  LN1 stats feature-major via PE column-sum matmuls (ones lhsT) over xT and
  xT^2; stats math on a [16,128] layout; scale/bias rows broadcast from DRAM.
  xn materialized once in fp8e4 DoubleRow layout [128, 2, S]; Q/K/V/Wo/FFN1
  run as fp8 DoubleRow matmuls (2x PE throughput). Scores stay bf16
  (contraction is only 64). Softmax exp is split across the Scalar engine
  (exact Exp with fp8 output) and the Vector engine (Schraudolph bit-trick:
  fp8 bits = s*8*log2e + 56 written through an int8 bitcast), alternating
  per key-block so neither engine is the bottleneck. attn@V runs fp8
  DoubleRow with a ones-column in V so the softmax denominator falls out of
  accumulator row 64; normalization multiplies by the broadcast reciprocal
  (x64 so o fits fp8). Wo runs twice (token- and feature-major, both fp8).
  FFN1 fp8 with per-column absmax weight quant (descale folded into relu);
  FFN2 bf16 for accuracy; final residual + store.
"""

import sys
import numpy as np

sys.path.insert(0, "/opt/trn_rl_repo")

B, S, D = 4, 2048, 512
H, DK, DFF = 8, 64, 2048
SQ = S // 2
P = 128
FD = 512
EPS = 1e-6
NKT = D // P          # 4  feature tiles
NST = S // P          # 16 sequence tiles
NSQT = SQ // P        # 8  own-query tiles
NMT = DFF // P        # 16 ffn tiles
SCL = 128.0           # fp8 weight prescale
OSC = 64.0            # fp8 o prescale
A_EXP8 = float(8.0 / np.log(2.0))
B_EXP8 = 55.57        # 7*8 - 0.43 schraudolph shift (fp8 bits)

_CACHE = {}
_TRACE = {"trace": False, "trace_cores": None}
_LAST = {"res": None}


def _np_reference(x, src_mask, Wq, bq, Wk, bk, Wv, bv, Wo, bo,
                  W1, b1, W2, b2, g1, be1, g2, be2):
    """Faithful numpy fallback (used only for off-nominal inputs)."""
    x = np.asarray(x, np.float32)

    def ln(t, g, be):
        m = t.mean(-1, keepdims=True)
        var = ((t - m) ** 2).sum(-1, keepdims=True) / (t.shape[-1] - 1)
        return g * (t - m) / (np.sqrt(var) + EPS) + be

    Bv, Sv, _ = x.shape
    xn = ln(x, g1, be1)
    q = (xn @ Wq + bq).reshape(Bv, Sv, H, DK).transpose(0, 2, 1, 3)
    k = (xn @ Wk + bk).reshape(Bv, Sv, H, DK).transpose(0, 2, 1, 3)
    v = (xn @ Wv + bv).reshape(Bv, Sv, H, DK).transpose(0, 2, 1, 3)
    s = np.einsum("bhqd,bhkd->bhqk", q, k) / np.float32(np.sqrt(DK))
    s = np.where(np.asarray(src_mask) == 0, np.float32(-1e9), s)
    s = s - s.max(-1, keepdims=True)
    p = np.exp(s)
    p = p / p.sum(-1, keepdims=True)
    o = np.einsum("bhqk,bhkd->bhqd", p, v)
    o = o.transpose(0, 2, 1, 3).reshape(Bv, Sv, D)
    x = x + o @ Wo + bo
    xn = ln(x, g2, be2)
    return (x + np.maximum(xn @ W1 + b1, 0.0) @ W2 + b2).astype(np.float32)


def _build(g1, be1, g2, be2):
    import concourse.bass as bass
    import concourse.tile as tile
    from concourse import bacc, mybir
    from concourse.masks import make_identity
    from contextlib import ExitStack

    F32 = mybir.dt.float32
    F32R = mybir.dt.float32r
    BF16 = mybir.dt.bfloat16
    F8 = mybir.dt.float8e4
    I8 = mybir.dt.int8
    AF = mybir.ActivationFunctionType
    MUL = mybir.AluOpType.mult
    ADD = mybir.AluOpType.add
    MAX = mybir.AluOpType.max
    DR = mybir.MatmulPerfMode.DoubleRow

    nc = bacc.Bacc("TRN2", target_bir_lowering=False, debug=False)

    xT = nc.dram_tensor("xT", [D, S], BF16, kind="ExternalInput").ap()
    x_tok = nc.dram_tensor("x_tok", [SQ, D], F32, kind="ExternalInput").ap()
    Wq8 = nc.dram_tensor("Wq8", [2, P, 2, D], F8, kind="ExternalInput").ap()
    Wk8 = nc.dram_tensor("Wk8", [2, P, 2, D], F8, kind="ExternalInput").ap()
    Wv8 = nc.dram_tensor("Wv8", [2, P, 2, D], F8, kind="ExternalInput").ap()
    Wo8 = nc.dram_tensor("Wo8", [2, P, 2, D], F8, kind="ExternalInput").ap()
    W18 = nc.dram_tensor("W18", [2, P, 2, DFF], F8, kind="ExternalInput").ap()
    ds1 = nc.dram_tensor("ds1", [NMT, P], F32, kind="ExternalInput").ap()
    W2b = nc.dram_tensor("W2b", [DFF, D], BF16, kind="ExternalInput").ap()

    out = nc.dram_tensor("out", [SQ, D], F32, kind="ExternalOutput").ap()
    scr_st = nc.dram_tensor("scr_st", [2, S], F32)
    scr_r1 = nc.dram_tensor("scr_r1", [2, S], BF16)
    import os
    KDBG = int(os.environ.get("KDBG", "0"))
    if KDBG:
        d_kT = nc.dram_tensor("d_kT", [P, S], BF16, kind="ExternalOutput").ap()
        d_qT = nc.dram_tensor("d_qT", [P, SQ], BF16, kind="ExternalOutput").ap()
        d_vo8 = nc.dram_tensor("d_vo8", [P, 8, 2, H * (DK + 2)], F8,
                               kind="ExternalOutput").ap()
        d_pg = nc.dram_tensor("d_pg", [P, 8, 2 * FD], F8,
                              kind="ExternalOutput").ap()
        d_oT8 = nc.dram_tensor("d_oT8", [P, 2, FD], F8,
                               kind="ExternalOutput").ap()
        d_x2 = nc.dram_tensor("d_x2", [P, D], F32, kind="ExternalOutput").ap()
        d_xn2 = nc.dram_tensor("d_xn2", [P, 2, SQ], F8,
                               kind="ExternalOutput").ap()
        d_scb = nc.dram_tensor("d_scb", [P, S], BF16,
                               kind="ExternalOutput").ap()
        d_bib = nc.dram_tensor("d_bib", [P, S], BF16,
                               kind="ExternalOutput").ap()
        d_ff = nc.dram_tensor("d_ff", [P, FD], BF16,
                              kind="ExternalOutput").ap()
        d_s2b = nc.dram_tensor("d_s2b", [P, FD], F32,
                               kind="ExternalOutput").ap()
        d_b2b = nc.dram_tensor("d_b2b", [P, FD], F32,
                               kind="ExternalOutput").ap()
        d_x2T = nc.dram_tensor("d_x2T", [P, FD], F32,
                               kind="ExternalOutput").ap()
        d_rw2 = nc.dram_tensor("d_rw2", [4, P], BF16,
                               kind="ExternalOutput").ap()
        d_xn2b = nc.dram_tensor("d_xn2b", [P, 2, SQ], F8,
                                kind="ExternalOutput").ap()

    scr_s2 = nc.dram_tensor("scr_s2", [NSQT, P], F32)
    scr_b2 = nc.dram_tensor("scr_b2", [NSQT, P], F32)
    scr_dn = nc.dram_tensor("scr_dn", [16, FD], F32R)
    scr_dn2 = nc.dram_tensor("scr_dn2", [16, FD], F32R)

    def bcast_row(src_dram, nfree):
        return bass.AP(tensor=src_dram.tensor, offset=src_dram.offset,
                       ap=[[0, P], [1, nfree]])

    with tile.TileContext(nc) as tc, ExitStack() as OU:
        res = OU.enter_context(tc.tile_pool(name="res", bufs=1))

        ident = res.tile([P, P], F32, name="ident")
        make_identity(nc, ident)
        dnT = [res.tile([65, FD], F32R, name=f"dnT{t}") for t in range(8)]
        # fp8 o in DoubleRow layout: oT8[qc][g][:, i, :] holds o-features
        # g*256+i*128+p for query chunk qc
        oT8 = [[res.tile([P, 2, FD], F8, name=f"oT8_{qc}_{g}") for g in range(2)]
               for qc in range(2)]
        xT_t = [res.tile([P, S], BF16, name=f"xT{j}") for j in range(NKT)]
        xre = [res.tile([P, D], F32, name=f"xre{sq}") for sq in range(NSQT)]

        with ExitStack() as QK:
            qkv = QK.enter_context(tc.tile_pool(name="qkv", bufs=1))
            qTz = [[qkv.tile([P, SQ], BF16, name=f"qTz{j}_{h01}")
                    for h01 in range(2)] for j in range(NKT)]
            kT = [qkv.tile([P, S], BF16, name=f"kT{j}") for j in range(NKT)]
            vo8 = qkv.tile([P, 8, 2, H * (DK + 2)], F8, name="vo8")

            # ================= phase 1: LN1 stats + QKV =================
            with ExitStack() as P1:
                p1 = P1.enter_context(tc.tile_pool(name="p1", bufs=1))
                p1s = P1.enter_context(tc.tile_pool(name="p1s", bufs=3))
                ps_qkv = P1.enter_context(
                    tc.tile_pool(name="ps_qkv", bufs=6, space="PSUM"))

                for ch in range(2):
                    for j in range(NKT):
                        eng = nc.sync if j % 2 == 0 else nc.scalar
                        eng.dma_start(
                            xT_t[j][:, ch * SQ:(ch + 1) * SQ],
                            xT[j * P:(j + 1) * P, ch * SQ:(ch + 1) * SQ])
                Wq8_t, Wk8_t, Wv8_t = [], [], []
                for g in range(2):
                    w = p1.tile([P, 2, D], F8, name=f"Wk8_{g}")
                    nc.gpsimd.dma_start(w, Wk8[g])
                    Wk8_t.append(w)
                for g in range(2):
                    w = p1.tile([P, 2, D], F8, name=f"Wq8_{g}")
                    nc.gpsimd.dma_start(w, Wq8[g])
                    Wq8_t.append(w)
                for g in range(2):
                    w = p1.tile([P, 2, D], F8, name=f"Wv8_{g}")
                    nc.gpsimd.dma_start(w, Wv8[g])
                    Wv8_t.append(w)

                ones_bf = p1.tile([P, P], BF16, name="ones_bf")
                nc.vector.memset(ones_bf, 1.0)

                # colsum stats: sum x and sum x^2 over features, per token
                xsq = [p1.tile([P, S], BF16, name=f"xsq{j}") for j in range(NKT)]
                for j in range(NKT):
                    if j % 2 == 0:
                        nc.vector.tensor_mul(xsq[j], xT_t[j], xT_t[j])
                    else:
                        nc.scalar.square(xsq[j], xT_t[j])
                st_sa = p1.tile([1, S], F32, name="st_sa")
                st_sq = p1.tile([1, S], F32, name="st_sq")
                for c in range(4):
                    ps1 = ps_qkv.tile([P, FD], F32, name=f"s1_{c}", tag="qk")
                    ps2 = ps_qkv.tile([P, FD], F32, name=f"s2_{c}", tag="qk")
                    for j in range(NKT):
                        nc.tensor.matmul(ps1, ones_bf,
                                         xT_t[j][:, c * FD:(c + 1) * FD],
                                         start=(j == 0), stop=(j == NKT - 1))
                    for j in range(NKT):
                        nc.tensor.matmul(ps2, ones_bf,
                                         xsq[j][:, c * FD:(c + 1) * FD],
                                         start=(j == 0), stop=(j == NKT - 1))
                    nc.vector.tensor_copy(st_sa[0:1, c * FD:(c + 1) * FD],
                                          ps1[0:1, :])
                    nc.scalar.copy(st_sq[0:1, c * FD:(c + 1) * FD],
                                   ps2[0:1, :])
                nc.sync.dma_start(scr_st.ap()[0:1, :], st_sa)
                nc.sync.dma_start(scr_st.ap()[1:2, :], st_sq)

                def row16(scr, row):
                    return bass.AP(tensor=scr.ap().tensor,
                                   offset=scr.ap().offset + row * S,
                                   ap=[[P, 16], [1, P]])

                s16a = p1.tile([16, P], F32, name="s16a")
                s16b = p1.tile([16, P], F32, name="s16b")
                nc.sync.dma_start(s16a, row16(scr_st, 0))
                nc.sync.dma_start(s16b, row16(scr_st, 1))
                m16 = p1.tile([16, P], F32, name="m16")
                t16 = p1.tile([16, P], F32, name="t16")
                sd16 = p1.tile([16, P], F32, name="sd16")
                sc16 = p1.tile([16, P], BF16, name="sc16")
                bi16 = p1.tile([16, P], BF16, name="bi16")
                nc.vector.tensor_scalar_mul(m16, s16a, 1.0 / D)
                nc.vector.tensor_mul(t16, m16, s16a)
                nc.vector.tensor_sub(t16, s16b, t16)
                nc.scalar.activation(sd16, t16, AF.Sqrt, bias=0.0,
                                     scale=1.0 / (D - 1))
                nc.vector.tensor_scalar_add(sd16, sd16, EPS)
                nc.vector.reciprocal(sd16, sd16)
                nc.vector.tensor_scalar_mul(sc16, sd16, float(g1))
                nc.vector.tensor_mul(t16, m16, sc16)
                nc.vector.tensor_scalar(bi16, t16, -1.0, float(be1),
                                        op0=MUL, op1=ADD)

                def row16w(scr, row):
                    return bass.AP(tensor=scr.ap().tensor,
                                   offset=scr.ap().offset + row * S,
                                   ap=[[P, 16], [1, P]])

                nc.sync.dma_start(row16w(scr_r1, 0), sc16)
                nc.sync.dma_start(row16w(scr_r1, 1), bi16)
                scale_b = p1.tile([P, S], BF16, name="scale_b")
                bias_b = p1.tile([P, S], BF16, name="bias_b")
                nc.sync.dma_start(scale_b, bcast_row(scr_r1.ap()[0:1, :], S))
                nc.sync.dma_start(bias_b, bcast_row(scr_r1.ap()[1:2, :], S))

                # materialize xn in fp8 DoubleRow layout
                xq8 = [p1.tile([P, 2, S], F8, name=f"xq8_{g}") for g in range(2)]
                for kt in range(NKT):
                    g, i = kt // 2, kt % 2
                    t = p1s.tile([P, S], BF16, name="xnt", tag=f"xnt{kt % 2}")
                    nc.vector.tensor_mul(t, xT_t[kt], scale_b)
                    nc.vector.tensor_add(xq8[g][:, i, :], t, bias_b)

                for j in range(NKT):
                    nc.vector.memset(qTz[j][0], 0.0)
                    nc.vector.memset(qTz[j][1], 0.0)

                # K (full seq) then Q (own half), feature-major, fp8 DoubleRow
                fix_i = 0
                qk_work = []
                for j in range(NKT):
                    for sc in range(4):
                        qk_work.append(("k", j, sc))
                    for sc in range(2):
                        qk_work.append(("q", j, sc))
                    for st in range(4 * j, 4 * j + 4):
                        qk_work.append(("v", j, st))
                for (kind, j, sc) in qk_work:
                    ps = ps_qkv.tile([P, FD], F32, name="ps_q", tag="qk")
                    if kind == "v":
                        st = sc
                        for g in range(2):
                            nc.tensor.matmul(
                                ps, xq8[g][:, :, st * P:(st + 1) * P],
                                Wv8_t[g], start=(g == 0), stop=(g == 1),
                                perf_mode=DR)
                        vv = vo8[:, st // 2, st % 2, :].rearrange(
                            "p (h c) -> p h c", c=DK + 2)
                        nc.scalar.mul(vv[:, :, 0:DK],
                                      ps.rearrange("p (h c) -> p h c", c=DK),
                                      1.0 / SCL)
                        nc.vector.memset(vv[:, :, DK:DK + 2], 1.0)
                        continue
                    Wt = Wk8_t if kind == "k" else Wq8_t
                    for g in range(2):
                        nc.tensor.matmul(
                            ps, Wt[g][:, :, j * P:(j + 1) * P],
                            xq8[g][:, :, sc * FD:(sc + 1) * FD],
                            start=(g == 0), stop=(g == 1),
                            perf_mode=DR)
                    if kind == "k":
                        dst = kT[j][:, sc * FD:(sc + 1) * FD]
                        if fix_i % 2 == 0:
                            nc.vector.tensor_scalar_mul(dst, ps, 1.0 / SCL)
                        else:
                            nc.scalar.mul(dst, ps, 1.0 / SCL)
                    else:
                        for h01 in range(2):
                            bp = 64 * h01
                            dst = qTz[j][h01][bp:bp + DK,
                                              sc * FD:(sc + 1) * FD]
                            if (fix_i + h01) % 2 == 0:
                                nc.vector.tensor_scalar_mul(
                                    dst, ps[bp:bp + DK, :], 1.0 / SCL)
                            else:
                                nc.scalar.mul(dst, ps[bp:bp + DK, :],
                                              1.0 / SCL)
                    fix_i += 1


            if KDBG:
                nc.sync.dma_start(d_kT, kT[0])
                nc.sync.dma_start(d_qT, qT[0])
                nc.sync.dma_start(d_vo8, vo8)
            # ========== phases 2+3: attention + FFN, per query-chunk ==========
            with ExitStack() as P2:
                pp = P2.enter_context(tc.tile_pool(name="pp", bufs=2))
                p2s = P2.enter_context(tc.tile_pool(name="p2s", bufs=2))
                p3 = P2.enter_context(tc.tile_pool(name="p3", bufs=1))
                p3s = P2.enter_context(tc.tile_pool(name="p3s", bufs=3))
                Wo8_t, W18_t, W2_t = [], [], []
                for g in range(2):
                    w = p3.tile([P, 2, D], F8, name=f"Wo8_{g}")
                    nc.gpsimd.dma_start(w, Wo8[g])
                    Wo8_t.append(w)
                for g in range(2):
                    w = p3.tile([P, 2, DFF], F8, name=f"W18_{g}")
                    nc.gpsimd.dma_start(w, W18[g])
                    W18_t.append(w)
                for m in range(NMT):
                    w = p3.tile([P, D], BF16, name=f"W2_{m}")
                    nc.gpsimd.dma_start(w, W2b[m * P:(m + 1) * P, :])
                    W2_t.append(w)
                ds1_sb = p3.tile([P, NMT], F32, name="ds1_sb")
                nc.sync.dma_start(ds1_sb, bass.AP(
                    tensor=ds1.tensor, offset=ds1.offset,
                    ap=[[1, P], [P, NMT]]))
                for sq in range(NSQT):
                    nc.gpsimd.dma_start(xre[sq], x_tok[sq * P:(sq + 1) * P, :])
                x2tok = [p3.tile([P, D], F32, name=f"x2t{sq}")
                         for sq in range(NSQT)]
                mv2 = p3.tile([P, 2, NSQT], F32, name="mv2")
                xn2_8 = [p3.tile([P, 2, SQ], F8, name=f"xn2_8_{g}")
                         for g in range(2)]
                oT_bf = [p3.tile([P, FD], BF16, name=f"oTb_{hp}")
                         for hp in range(4)]

                def attention(qc, ps_sc, ps_acc):
                    for hp in range(4):
                        accs = {}
                        for h01 in range(2):
                            accs[h01] = ps_acc.tile(
                                [DK + 2, FD], F32,
                                name=f"acc{h01}", tag=f"acc{h01}")
                        pgs = {}
                        for g in range(8):
                            sgs = {}
                            for half in range(2):
                                kt = 2 * g + half
                                for h01 in range(2):
                                    sgc = ps_sc.tile([P, FD], F32, name="sg",
                                                     tag="sg", bufs=4)
                                    nc.tensor.matmul(
                                        sgc, kT[hp][:, kt * P:(kt + 1) * P],
                                        qTz[hp][h01][:,
                                                     qc * FD:(qc + 1) * FD])
                                    sgs[(half, h01)] = sgc
                            for h01 in range(2):
                                pgt = pp.tile([P, 2 * FD], F8, name="pg",
                                              tag="pg", bufs=16)
                                pgs[(g, h01)] = pgt
                                for half in range(2):
                                    pgv = pgt[:, half * FD:(half + 1) * FD]
                                    if (half + h01) % 2 == 0:
                                        nc.scalar.activation(
                                            pgv, sgs[(half, h01)], AF.Exp)
                                    else:
                                        nc.vector.tensor_scalar(
                                            pgv.bitcast(I8),
                                            sgs[(half, h01)], A_EXP8, B_EXP8,
                                            op0=MUL, op1=ADD)
                        for h01 in range(2):
                            h = 2 * hp + h01
                            for g in range(8):
                                nc.tensor.matmul(
                                    accs[h01],
                                    vo8[:, g, :,
                                        h * (DK + 2):(h + 1) * (DK + 2)],
                                    pgs[(g, h01)].rearrange(
                                        "p (a b) -> p a b", b=FD),
                                    start=(g == 0), stop=(g == 7),
                                    perf_mode=DR)
                        t = qc * 4 + hp
                        for h01 in range(2):
                            acc = accs[h01]
                            nc.scalar.copy(
                                dnT[t][64 * h01:64 * h01 + 1, :],
                                acc[DK:DK + 1, :])
                            if h01 == 0:
                                nc.scalar.copy(
                                    oT_bf[hp][0:DK, :], acc[0:DK, :])
                            else:
                                nc.scalar.copy(
                                    oT_bf[hp][DK:P, :], acc[0:DK, :])
                            nc.sync.dma_start(
                                scr_dn.ap()[2 * t + h01:2 * t + h01 + 1, :],
                                dnT[t][64 * h01:64 * h01 + 1, :])

                def normalize(qc):
                    rcp = p2s.tile([64, 64], F32R, name="rcp", tag="rcp")
                    nc.sync.dma_start(rcp, bass.AP(
                        tensor=scr_dn.ap().tensor, offset=qc * 8 * FD,
                        ap=[[64, 64], [1, 64]]))
                    with nc.allow_low_precision(reason="denom recip"):
                        nc.vector.reciprocal(rcp, rcp)
                    nc.vector.tensor_scalar_mul(rcp, rcp, OSC)
                    nc.sync.dma_start(bass.AP(
                        tensor=scr_dn2.ap().tensor, offset=qc * 8 * FD,
                        ap=[[64, 64], [1, 64]]), rcp)
                    for hp in range(4):
                        t = qc * 4 + hp
                        rb_sb = p2s.tile([P, FD], F32R, name="rbs", tag="rbs")
                        for par in range(2):
                            row = scr_dn2.ap()[2 * t + par:2 * t + par + 1, :]
                            nc.sync.dma_start(
                                rb_sb[64 * par:64 * par + 64, :],
                                bass.AP(tensor=row.tensor, offset=row.offset,
                                        ap=[[0, 64]] + row.ap[1:]))
                        g, i = hp // 2, hp % 2
                        nc.vector.tensor_mul(oT8[qc][g][:, i, :],
                                             oT_bf[hp], rb_sb)
                    if KDBG and qc == 0:
                        nc.sync.dma_start(d_oT8, oT8[0][0])

                def wo_stats(qc, psp):
                    # Wo token-major + residual + LN2 stats
                    for sl in range(4):
                        sq = qc * 4 + sl
                        ps = psp.tile([P, D], F32, name="ps_wo", tag="ffn")
                        for g in range(2):
                            nc.tensor.matmul(
                                ps, oT8[qc][g][:, :, sl * P:(sl + 1) * P],
                                Wo8_t[g], start=(g == 0), stop=(g == 1),
                                perf_mode=DR)
                        nc.vector.scalar_tensor_tensor(
                            x2tok[sq], ps, 1.0 / (SCL * OSC), xre[sq],
                            op0=MUL, op1=ADD)
                        st6b = p3s.tile([P, 6], F32, name="st6b", tag="st6b")
                        nc.vector.bn_stats(st6b, x2tok[sq])
                        nc.vector.bn_aggr(mv2[:, :, sq:sq + 1], st6b)
                        if KDBG and sq == 0:
                            nc.sync.dma_start(d_x2, x2tok[0])

                def ln2_ffn(qc, psp):
                    # LN2 scale/bias rows for this half
                    sc2 = p3s.tile([P, 4], F32, name="sc2", tag="ln2")
                    bi2 = p3s.tile([P, 4], F32, name="bi2", tag="ln2")
                    std2 = p3s.tile([P, 4], F32, name="std2", tag="ln2")
                    nc.scalar.activation(std2, mv2[:, 1, 4 * qc:4 * qc + 4],
                                         AF.Sqrt, bias=0.0,
                                         scale=float(D) / (D - 1))
                    nc.vector.tensor_scalar_add(std2, std2, EPS)
                    nc.vector.reciprocal(std2, std2)
                    nc.vector.tensor_scalar_mul(sc2, std2, float(g2))
                    nc.vector.tensor_mul(std2, mv2[:, 0, 4 * qc:4 * qc + 4], sc2)
                    nc.vector.tensor_scalar(bi2, std2, -1.0, float(be2),
                                            op0=MUL, op1=ADD)
                    s2b = p2s.tile([P, FD], F32, name="s2b", tag="s2b")
                    b2b = p2s.tile([P, FD], F32, name="b2b", tag="s2b")
                    for src, scr, dst in ((sc2, scr_s2, s2b),
                                          (bi2, scr_b2, b2b)):
                        tp2 = psp.tile([4, P], F32, name="tp2", tag="ffn")
                        nc.tensor.transpose(tp2, src, ident)
                        rw2 = p3s.tile([4, P], F32, name="rw2", tag="rw2")
                        nc.vector.tensor_copy(rw2, tp2)
                        nc.sync.dma_start(scr.ap()[4 * qc:4 * qc + 4, :], rw2)
                        nc.sync.dma_start(dst, bass.AP(
                            tensor=scr.ap().tensor,
                            offset=scr.ap().offset + qc * 4 * P,
                            ap=[[0, P], [1, FD]]))
                    if KDBG and qc == 0:
                        nc.sync.dma_start(d_s2b, s2b)
                        nc.sync.dma_start(d_b2b, b2b)
                    # Wo feature-major + residual in x^T layout + LN2 apply
                    for j in range(NKT):
                        ps = psp.tile([P, FD], F32, name="ps_woT", tag="ffn")
                        for g in range(2):
                            nc.tensor.matmul(ps, Wo8_t[g][:, :, j * P:(j + 1) * P],
                                             oT8[qc][g], start=(g == 0),
                                             stop=(g == 1), perf_mode=DR)
                        x2T = p3s.tile([P, FD], F32, name="x2T", tag="x2T",
                                       bufs=2)
                        nc.vector.scalar_tensor_tensor(
                            x2T, ps, 1.0 / (SCL * OSC),
                            xT_t[j][:, qc * FD:(qc + 1) * FD],
                            op0=MUL, op1=ADD)
                        if KDBG and qc == 0 and j == 0:
                            nc.sync.dma_start(d_x2T, x2T)
                        t2 = p3s.tile([P, FD], F32, name="t2", tag="t2",
                                      bufs=2)
                        eng = nc.gpsimd if (qc == 0 and j % 2 == 1) \
                            else nc.vector
                        eng.tensor_mul(t2, x2T, s2b)
                        g2i, i2 = j // 2, j % 2
                        eng.tensor_add(
                            xn2_8[g2i][:, i2, qc * FD:(qc + 1) * FD], t2, b2b)
                    if KDBG and qc == 1:
                        nc.sync.dma_start(d_xn2, xn2_8[0])
                    if KDBG and qc == 0:
                        nc.sync.dma_start(d_xn2b, xn2_8[0])
                    # FFN1 fp8 DoubleRow with per-column descale + relu
                    ffb = []
                    for mt in range(NMT):
                        ps = psp.tile([P, FD], F32, name="ps_f1", tag="ffn")
                        for g in range(2):
                            nc.tensor.matmul(
                                ps, W18_t[g][:, :, mt * P:(mt + 1) * P],
                                xn2_8[g][:, :, qc * FD:(qc + 1) * FD],
                                start=(g == 0), stop=(g == 1), perf_mode=DR)
                        ff = p3s.tile([P, FD], BF16, name=f"ff{mt}",
                                      tag=f"ff{mt}", bufs=1)
                        nc.scalar.activation(
                            ff, ps, AF.Relu, bias=0.0,
                            scale=ds1_sb[:, mt:mt + 1])
                        ffb.append(ff)
                        if KDBG and qc == 0 and mt == 0:
                            nc.sync.dma_start(d_ff, ff)
                    # FFN2 bf16
                    for sl in range(4):
                        sq = qc * 4 + sl
                        ps = psp.tile([P, D], F32, name="ps_f2", tag="ffn")
                        for mt in range(NMT):
                            nc.tensor.matmul(ps, ffb[mt][:, sl * P:(sl + 1) * P],
                                             W2_t[mt], start=(mt == 0),
                                             stop=(mt == NMT - 1))
                        ot = p3s.tile([P, D], F32, name="ot", tag="ot",
                                      bufs=2)
                        nc.vector.tensor_add(ot, ps, x2tok[sq])
                        nc.sync.dma_start(out[sq * P:(sq + 1) * P, :], ot)

                with ExitStack() as PA:
                    ps_sc = PA.enter_context(
                        tc.tile_pool(name="ps_sc", bufs=2, space="PSUM"))
                    ps_acc = PA.enter_context(
                        tc.tile_pool(name="ps_acc", bufs=1, space="PSUM"))
                    ps_big = PA.enter_context(
                        tc.tile_pool(name="ps_big", bufs=2, space="PSUM"))
                    attention(0, ps_sc, ps_acc)
                    normalize(0)
                    wo_stats(0, ps_big)
                    attention(1, ps_sc, ps_acc)
                    ln2_ffn(0, ps_big)
                    normalize(1)
                    wo_stats(1, ps_big)
                    ln2_ffn(1, ps_big)

    nc.compile()
    return nc


def _fast_path_ok(inputs):
    if not np.all(np.asarray(inputs["src_mask"]) != 0):
        return False
    for b in ("bq", "bk", "bv", "bo", "b1", "b2"):
        if np.any(np.asarray(inputs[b]) != 0):
            return False
    return True


def _pack_dr(W, scale):
    """Pack [K, M] weight into fp8 DoubleRow layout [K//256, 128, 2, M]."""
    import ml_dtypes
    K, M = W.shape
    Wp = (np.asarray(W, np.float32) * scale).reshape(K // 256, 2, P, M)
    Wp = Wp.transpose(0, 2, 1, 3)
    return np.ascontiguousarray(Wp.astype(ml_dtypes.float8_e4m3))


def kernel(**inputs):
    x = np.ascontiguousarray(np.asarray(inputs["x"], np.float32))
    g1 = float(np.asarray(inputs["g1"]))
    be1 = float(np.asarray(inputs["be1"]))
    g2 = float(np.asarray(inputs["g2"]))
    be2 = float(np.asarray(inputs["be2"]))

    if not _fast_path_ok(inputs):
        return _np_reference(**{k: np.asarray(v) for k, v in inputs.items()})

    from concourse.bass_utils import run_bass_kernel_spmd

    key = (g1, be1, g2, be2)
    if key not in _CACHE:
        _CACHE[key] = _build(*key)
    nc = _CACHE[key]

    import ml_dtypes
    BF = ml_dtypes.bfloat16
    scale = np.float32(1.0 / np.sqrt(DK))
    Wq8 = _pack_dr(inputs["Wq"], SCL * scale)
    Wk8 = _pack_dr(inputs["Wk"], SCL)
    Wv8 = _pack_dr(inputs["Wv"], SCL)
    Wo8 = _pack_dr(inputs["Wo"], SCL)
    W1 = np.asarray(inputs["W1"], np.float32)
    scl1 = 240.0 / np.abs(W1).max(0)
    W18 = _pack_dr(W1 * scl1, 1.0)
    ds1 = np.ascontiguousarray((1.0 / scl1).reshape(NMT, P).astype(np.float32))
    W2b = np.ascontiguousarray(np.asarray(inputs["W2"], np.float32).astype(BF))

    in_maps = []
    for c in range(8):
        b, hh = c // 2, c % 2
        if hh == 0:
            xp = x[b]
        else:
            xp = np.concatenate([x[b, SQ:], x[b, :SQ]], axis=0)
        xp = np.ascontiguousarray(xp)
        in_maps.append(dict(
            xT=np.ascontiguousarray(xp.T.astype(BF)),
            x_tok=np.ascontiguousarray(xp[:SQ]),
            Wq8=Wq8, Wk8=Wk8, Wv8=Wv8, Wo8=Wo8, W18=W18, ds1=ds1, W2b=W2b))

    res = run_bass_kernel_spmd(nc, in_maps, core_ids=list(range(8)),
                               trace=_TRACE["trace"],
                               trace_cores=_TRACE["trace_cores"])
    _LAST["res"] = res

    full = np.empty((B, S, D), np.float32)
    for c in range(8):
        b, hh = c // 2, c % 2
        full[b, hh * SQ:(hh + 1) * SQ] = res.results[c]["out"]
    return full
